# revision 25
# baseline (speedup 1.0000x reference)
"""MLA (multi-headed latent attention) forward on 8 Trainium2 NeuronCores.

Sharding: data-parallel over batch (4) x tensor-parallel over heads (2):
core c handles batch c//2 with heads [16*(c%2), 16*(c%2)+16).
Each core computes a partial (H-dim) output contribution; host sums the
TP pair and stacks batches.

All matmuls run in bf16 (fp32 PSUM accumulation). Weights are host-
pre-tiled into the exact SBUF layouts so every DMA is contiguous.
LN means are folded into the fused projection as extra weight columns.

Schedule (the point of this structure is to keep the PE gap-free so the
HAM clock gate stays at 2.4 GHz):
- phase-1 m-tile order pe, qa x6, kv x4 so both TP AllGathers launch
  early and complete while the PE chews the kv tiles;
- mean subtraction happens on the raw bf16 projections as soon as the
  inputs exist; the qa rstd is folded into the qb-projection copies so
  the sqrt chain is never on the PE critical path;
- the causal mask is a third accumulating matmul on diagonal score
  blocks (exp of -30000 -> 0), removing the vector hop from the
  softmax chain: PE -> Act -> PE;
- the two heads of each group are interleaved block-by-block and
  sum/pv matmuls are software-pipelined one block behind the scores,
  so there is always independent PE work while Act computes exp.
Layout is feature-major ("T" = [feature, token]) everywhere except v
(token-major for the PV matmul).
"""

import numpy as np
import ml_dtypes
import concourse.bass as bass
import concourse.mybir as mybir
import concourse.tile as tile
from concourse import bacc
from concourse import bass_utils

F32 = mybir.dt.float32
BF16 = mybir.dt.bfloat16
AX = mybir.AxisListType
OP = mybir.AluOpType
AF = mybir.ActivationFunctionType
NPBF = ml_dtypes.bfloat16

B, S, H, NH = 4, 1024, 4096, 32
QL, KVL, RD, ND, VD = 1536, 512, 64, 128, 128
QHD = ND + RD  # 192
EPS = 1e-6
NCORES = 8
TP = 2                 # tensor-parallel ways (heads)
HPC = NH // TP         # 16 heads per core
G = 2                  # heads per group
NG = HPC // G          # 8 groups
TOKT = S // 128        # 8 token tiles
KH = H // 128          # 32 contraction tiles for H
NMT = 9                # phase-1 m-tiles/core: 1 pe+stats, 6 own qa, 2 own kv
OWN = 6                # qa m-tiles computed locally; partner half via AllGather
KVOWN = 2              # kv m-tiles computed locally; partner half via AllGather
CC_GROUPS = [[0, 1], [2, 3], [4, 5], [6, 7]]
SCALE = float(QHD) ** -0.5
MASKV = -30000.0

# rope feature permutation: pairs (d, d+32) land 16 lanes apart within a
# 32-partition quadrant so stream_shuffle can do rotate_half.
DIMS_PERM = np.array(
    list(range(0, 16)) + list(range(32, 48))
    + list(range(16, 32)) + list(range(48, 64)), dtype=np.int64)
SHUF_MASK = [(i + 16) % 32 for i in range(32)]

_NC_CACHE = {}


def _build_nc():
    nc = bacc.Bacc("TRN2", target_bir_lowering=False, debug=False)

    hs_d = nc.dram_tensor("hs_d", (128, KH, S), BF16, kind="ExternalInput").ap()
    wa_d = nc.dram_tensor("wa_d", (128, NMT, KH, 128), BF16, kind="ExternalInput").ap()
    wqb_d = nc.dram_tensor("wqb_d", (128, NG, 3, 12, 128), BF16, kind="ExternalInput").ap()
    wk_d = nc.dram_tensor("wk_d", (128, NG, 2, 4, 128), BF16, kind="ExternalInput").ap()
    wv_d = nc.dram_tensor("wv_d", (128, NG, 4, 256), BF16, kind="ExternalInput").ap()
    wo_d = nc.dram_tensor("wo_d", (128, KH, HPC, 128), BF16, kind="ExternalInput").ap()
    csq_d = nc.dram_tensor("csq_d", (128, S), BF16, kind="ExternalInput").ap()
    ssq_d = nc.dram_tensor("ssq_d", (128, S), BF16, kind="ExternalInput").ap()
    mask_d = nc.dram_tensor("mask_d", (128, 128), BF16, kind="ExternalInput").ap()
    id_d = nc.dram_tensor("id_d", (128, 128), BF16, kind="ExternalInput").ap()
    ones_d = nc.dram_tensor("ones_d", (128, 1), BF16, kind="ExternalInput").ap()
    outT = nc.dram_tensor("outT", (H, S), F32, kind="ExternalOutput").ap()
    stage_a = nc.dram_tensor("stage_a", (512, S), BF16, kind="Internal").ap()
    stage_b = nc.dram_tensor("stage_b", (258, S), BF16, kind="Internal").ap()
    stage_k = nc.dram_tensor("stage_k", (258, S), BF16, kind="Internal").ap()
    gath_a = nc.dram_tensor("gath_a", (2, 512, S), BF16, kind="Internal").ap()
    gath_b = nc.dram_tensor("gath_b", (2, 258, S), BF16, kind="Internal").ap()
    gath_k = nc.dram_tensor("gath_k", (2, 258, S), BF16, kind="Internal").ap()

    with tile.TileContext(nc) as tc:
        with tc.tile_pool(name="pers", bufs=1) as pers:
            # ---------------- persistent tiles ----------------
            q_anT = pers.tile([128, 12 * S], BF16)     # (q_a - m)^T  (1536, 1024)
            kv_cnT = pers.tile([128, 4 * S], BF16)     # LN(kv_c)^T (512, 1024)
            kpeT2 = pers.tile([128, S], BF16)          # roped k_pe^T, both halves
            csq_t = pers.tile([128, S], BF16)
            ssq_t = pers.tile([128, S], BF16)
            mask_t = pers.tile([128, 128], BF16)
            id_t = pers.tile([128, 128], BF16)
            ones_t = pers.tile([128, 1], BF16)
            mrow_qa = pers.tile([1, S], F32, name="mrow_qa")
            mrow_kv = pers.tile([1, S], F32, name="mrow_kv")
            sqrow_qa = pers.tile([1, S], F32, name="sqrow_qa")
            sqrow_kv = pers.tile([1, S], F32, name="sqrow_kv")
            prow_kv = pers.tile([1, S], F32, name="prow_kv")
            mb_qa = pers.tile([128, S], BF16, name="mb_qa")    # mean bcast
            mb_kv = pers.tile([128, S], BF16, name="mb_kv")
            rb_qa = pers.tile([128, S], F32, name="rb_qa")     # rstd bcast
            rb_kv = pers.tile([128, S], F32, name="rb_kv")
            vrow_qa = pers.tile([1, S], F32, name="vrow_qa")
            vrow_kv = pers.tile([1, S], F32, name="vrow_kv")
            # ones first (tiny, needed by the stats matmuls); mask/id via
            # the software-DGE pool queue (needed only by attention).
            nc.scalar.dma_start(out=ones_t[:, :], in_=ones_d)
            nc.gpsimd.dma_start(out=mask_t[:, :], in_=mask_d)
            nc.gpsimd.dma_start(out=id_t[:, :], in_=id_d)

            def rstd_chain(v1, sqrow, mrow, rb):
                # rows ([1,S]): var = E[x^2] - m^2; rstd = 1/sqrt(var+eps)
                nc.vector.tensor_tensor(out=v1[:, :], in0=mrow[:, :],
                                        in1=mrow[:, :], op=OP.mult)
                nc.vector.tensor_tensor(out=v1[:, :], in0=sqrow[:, :],
                                        in1=v1[:, :], op=OP.subtract)
                nc.vector.tensor_scalar_add(v1[:, :], v1[:, :], EPS)
                nc.scalar.sqrt(v1[:, :], v1[:, :])
                nc.vector.reciprocal_approx_fast(out=v1[:, :], in_=v1[:, :])
                nc.gpsimd.partition_broadcast(rb[:, :], v1[:, :])

            # ======== phase 1 (feature-major): X^T = Wa^T @ hs^T ====
            # m-tile order: pe+stats first (means), qa0-3 (first gather),
            # then the own kv tiles (kv gather hides under qa4/qa5), then
            # qa4/qa5 (second gather hides under the phase-2 qb work).
            M_TILES = ([("pe", 0)] + [("qa", i) for i in range(4)]
                       + [("kv", i) for i in range(KVOWN)]
                       + [("qa", i) for i in range(4, OWN)])

            with tc.tile_pool(name="hsp", bufs=1) as hsp, \
                 tc.tile_pool(name="p1wa", bufs=3) as p1wa, \
                 tc.tile_pool(name="sqp", bufs=2) as sqp, \
                 tc.tile_pool(name="rowp", bufs=1) as rowp, \
                 tc.tile_pool(name="p1ps", bufs=3, space="PSUM") as p1ps, \
                 tc.tile_pool(name="stps", bufs=1, space="PSUM") as stps:
                hst = [hsp.tile([128, 4, S], BF16, name=f"hst_{i}")
                       for i in range(8)]
                # evens on the Act queue, odds interleaved between the early
                # weight tiles on the SP queue; cos/sin trail the Act stream.
                for i in (0, 2, 4, 6):
                    nc.scalar.dma_start(out=hst[i][:, :, :],
                                        in_=hs_d[:, 4 * i:4 * (i + 1), :])
                nc.scalar.dma_start(out=csq_t[:, :], in_=csq_d)
                nc.scalar.dma_start(out=ssq_t[:, :], in_=ssq_d)

                # sum-of-squares stats accumulators (fp32 PSUM)
                stat = {("qa", 0): stps.tile([1, 512], F32, name="st_qa0"),
                        ("qa", 1): stps.tile([1, 512], F32, name="st_qa1"),
                        ("kv", 0): stps.tile([1, 512], F32, name="st_kv0"),
                        ("kv", 1): stps.tile([1, 512], F32, name="st_kv1")}

                def mean_bcast(kind, mrow, mb):
                    # f32 mean row -> bf16 row -> [128,S] broadcast
                    r16 = rowp.tile([1, S], BF16, name=f"r16_{kind}")
                    nc.vector.tensor_scalar_mul(r16[:, :], mrow[:, :], 1.0)
                    nc.gpsimd.partition_broadcast(mb[:, :], r16[:, :])

                def tile_epilogue(kind, mi, dest):
                    # Square + stats matmuls + staging for one m-tile.
                    # Deferred one m-tile so the in-order PE queue never
                    # waits on the Act-queue Square before the next chains.
                    nmt = OWN if kind == "qa" else KVOWN
                    sqt = sqp.tile([128, S], BF16, tag="sq")
                    nc.scalar.activation(sqt[:, :], dest, AF.Square)
                    for qh in range(2):
                        sl = slice(qh * 512, qh * 512 + 512)
                        nc.tensor.matmul(
                            stat[(kind, qh)][:, :], ones_t[:, :], sqt[:, sl],
                            start=(mi == 0), stop=(mi == nmt - 1))
                    if kind == "qa":
                        # stage raw tiles for the TP-pair exchange
                        stg = stage_a if mi < 4 else stage_b
                        ro = (mi if mi < 4 else mi - 4) * 128
                        nc.sync.dma_start(out=stg[ro:ro + 128, :], in_=dest)
                        if mi == 3:
                            nc.gpsimd.collective_compute(
                                "AllGather", OP.bypass,
                                replica_groups=CC_GROUPS,
                                ins=[stage_a], outs=[gath_a])
                        if mi == OWN - 1:
                            # own E[x^2] rows, staged bitcast + second gather
                            for qh in range(2):
                                sl = slice(qh * 512, qh * 512 + 512)
                                nc.vector.tensor_scalar_mul(
                                    sqrow_qa[:, sl], stat[("qa", qh)][:, :],
                                    1.0 / QL)
                            sq16 = sqrow_qa[:, :].bitcast(BF16)  # [1, 2048]
                            nc.sync.dma_start(out=stage_b[256:257, :],
                                              in_=sq16[:, 0:S])
                            nc.sync.dma_start(out=stage_b[257:258, :],
                                              in_=sq16[:, S:2 * S])
                            nc.gpsimd.collective_compute(
                                "AllGather", OP.bypass,
                                replica_groups=CC_GROUPS,
                                ins=[stage_b], outs=[gath_b])
                            # mean-subtract the own qa slices now (rstd is
                            # folded into the qb copies later)
                            for mj in range(OWN):
                                dsl = q_anT[:, mj * S:(mj + 1) * S]
                                nc.vector.tensor_tensor(
                                    out=dsl, in0=dsl, in1=mb_qa[:, :],
                                    op=OP.subtract)
                    else:  # kv: stage for exchange, mean-subtract in place
                        nc.sync.dma_start(
                            out=stage_k[mi * 128:(mi + 1) * 128, :], in_=dest)
                        if mi == KVOWN - 1:
                            for qh in range(2):
                                sl = slice(qh * 512, qh * 512 + 512)
                                nc.vector.tensor_scalar_mul(
                                    sqrow_kv[:, sl], stat[("kv", qh)][:, :],
                                    1.0 / KVL)
                            sqk16 = sqrow_kv[:, :].bitcast(BF16)
                            nc.sync.dma_start(out=stage_k[256:257, :],
                                              in_=sqk16[:, 0:S])
                            nc.sync.dma_start(out=stage_k[257:258, :],
                                              in_=sqk16[:, S:2 * S])
                            nc.gpsimd.collective_compute(
                                "AllGather", OP.bypass,
                                replica_groups=CC_GROUPS,
                                ins=[stage_k], outs=[gath_k])
                        nc.vector.tensor_tensor(out=dest, in0=dest,
                                                in1=mb_kv[:, :], op=OP.subtract)

                first_sync_hs = [False]
                pending = None

                for (kind, mi) in M_TILES:
                    tix = {"pe": 0, "qa": 1, "kv": 7}[kind] + mi
                    wt = p1wa.tile([128, KH, 128], BF16, tag="wa",
                                   name=f"wa_{kind}_{mi}")
                    nc.sync.dma_start(out=wt[:, :, :], in_=wa_d[:, tix, :, :])
                    if not first_sync_hs[0]:
                        # odd hs tiles follow the first weight tile on sync;
                        # they must all be emitted before the first m-tile's
                        # matmuls (deps follow emission order)
                        first_sync_hs[0] = True
                        for i in (1, 3, 5, 7):
                            nc.sync.dma_start(
                                out=hst[i][:, :, :],
                                in_=hs_d[:, 4 * i:4 * (i + 1), :])
                    if kind == "qa":
                        dest = q_anT[:, mi * S:(mi + 1) * S]
                    elif kind == "kv":
                        dest = kv_cnT[:, mi * S:(mi + 1) * S]
                    else:
                        dest = kpeT2[0:64, :]
                    for qh in range(2):
                        sl = slice(qh * 512, qh * 512 + 512)
                        ps = p1ps.tile([128, 512], F32, tag="p1")
                        for k in range(KH):
                            nc.tensor.matmul(
                                ps[:, :], wt[:, k, :], hst[k // 4][:, k % 4, sl],
                                start=(k == 0), stop=(k == KH - 1))
                        if kind == "pe":
                            nc.scalar.copy(dest[:, sl], ps[0:64, :])
                            # rows 64/96 carry sum(qa), sum(kv) over features
                            nc.vector.tensor_scalar_mul(
                                mrow_qa[:, sl], ps[64:65, :], 1.0 / QL)
                            nc.vector.tensor_scalar_mul(
                                mrow_kv[:, sl], ps[96:97, :], 1.0 / KVL)
                        else:
                            nc.scalar.copy(dest[:, sl], ps[:, :])
                    if kind == "pe":
                        # means -> bf16 broadcasts (early: deps ready now)
                        mean_bcast("qa", mrow_qa, mb_qa)
                        mean_bcast("kv", mrow_kv, mb_kv)
                        # rope k_pe rows 0:64, duplicate into 64:128 (the
                        # duplicate is a software-DGE copy; slack is huge)
                        kp_sh = sqp.tile([64, S], BF16, tag="kpsh", name="kpsh")
                        nc.vector.stream_shuffle(
                            kp_sh[:, :].bitcast(F32), kpeT2[0:64, :].bitcast(F32),
                            SHUF_MASK)
                        nc.vector.tensor_tensor(out=kp_sh[:, :], in0=kp_sh[:, :],
                                                in1=ssq_t[:64, :], op=OP.mult)
                        nc.vector.tensor_tensor(out=kpeT2[0:64, :],
                                                in0=kpeT2[0:64, :],
                                                in1=csq_t[:64, :], op=OP.mult)
                        nc.vector.tensor_tensor(out=kpeT2[0:64, :],
                                                in0=kpeT2[0:64, :],
                                                in1=kp_sh[:, :], op=OP.add)
                        nc.gpsimd.dma_start(out=kpeT2[64:128, :],
                                            in_=kpeT2[0:64, :])
                        continue
                    if pending is not None:
                        tile_epilogue(*pending)
                    pending = (kind, mi, dest)
                if pending is not None:
                    tile_epilogue(*pending)

                # ---- partner fetch: emitted after the kv weight stream so
                # the gather-wait can't head-of-line-block it on sync ----
                pid = nc.sync.partition_id()
                partner = 1 - (pid % 2)
                nc.sync.dma_start(
                    out=q_anT[:, OWN * S:10 * S]
                        .rearrange("p (k t) -> p k t", k=4),
                    in_=gath_a[partner, 0:512, :]
                        .rearrange("(k p) t -> p k t", p=128))
                nc.sync.dma_start(
                    out=q_anT[:, 10 * S:12 * S]
                        .rearrange("p (k t) -> p k t", k=2),
                    in_=gath_b[partner, 0:256, :]
                        .rearrange("(k p) t -> p k t", p=128))
                prow = rowp.tile([1, S], F32, name="prow")
                nc.sync.dma_start(
                    out=prow[:, :],
                    in_=gath_b[partner, 256:258, :].bitcast(F32))
                # partner-half mean subtraction + stats merge
                for mj in range(OWN, 12):
                    dsl = q_anT[:, mj * S:(mj + 1) * S]
                    nc.vector.tensor_tensor(out=dsl, in0=dsl, in1=mb_qa[:, :],
                                            op=OP.subtract)
                nc.vector.tensor_tensor(out=sqrow_qa[:, :], in0=sqrow_qa[:, :],
                                        in1=prow[:, :], op=OP.add)
                rstd_chain(vrow_qa, sqrow_qa, mrow_qa, rb_qa)

            # ======== phase 2 + 3 ========
            with tc.tile_pool(name="otp", bufs=1) as otp:
                oT = otp.tile([128, HPC * S], BF16)    # normalized o^T
                with tc.tile_pool(name="gq2", bufs=3) as gqp, \
                     tc.tile_pool(name="wqp", bufs=2) as wqp, \
                     tc.tile_pool(name="wop", bufs=2) as wop, \
                     tc.tile_pool(name="op", bufs=2) as outp, \
                     tc.tile_pool(name="gkv", bufs=3) as gkvp, \
                     tc.tile_pool(name="wk", bufs=3) as wkp, \
                     tc.tile_pool(name="wv", bufs=2) as wvp, \
                     tc.tile_pool(name="rshp", bufs=1) as rshp, \
                     tc.tile_pool(name="pp", bufs=2) as ppool, \
                     tc.tile_pool(name="denp", bufs=1) as denp, \
                     tc.tile_pool(name="pjps", bufs=2, space="PSUM") as pjps, \
                     tc.tile_pool(name="sps", bufs=1, space="PSUM") as sps, \
                     tc.tile_pool(name="ops", bufs=1, space="PSUM") as ops, \
                     tc.tile_pool(name="smps", bufs=1, space="PSUM") as smps:

                    def qT_dma(g):
                        wt = wqp.tile([128, 3, 12, 128], BF16, tag="wqb",
                                      name=f"wqb_{g}")
                        nc.sync.dma_start(out=wt[:, :, :, :],
                                          in_=wqb_d[:, g, :, :, :])
                        return wt

                    def qT_mm(g, wt):
                        # q^T for this group: 2 nope m-tiles + 1 pe pair;
                        # the qa rstd is folded into the PSUM->SBUF copies.
                        qT = gqp.tile([128, 3 * S], BF16, tag="qT",
                                      name=f"qT_{g}")
                        for m in range(3):
                            for qh in range(2):
                                sl = slice(qh * 512, qh * 512 + 512)
                                ps = pjps.tile([128, 512], F32, tag="pj")
                                for k in range(12):
                                    nc.tensor.matmul(
                                        ps[:, :], wt[:, m, k, :],
                                        q_anT[:, k * S + qh * 512:
                                              k * S + qh * 512 + 512],
                                        start=(k == 0), stop=(k == 11))
                                nc.vector.tensor_tensor(
                                    out=qT[:, m * S + qh * 512:
                                           m * S + qh * 512 + 512],
                                    in0=ps[:, :], in1=rb_qa[:, sl], op=OP.mult)
                        # rope the pe tile (m=2): rows 0:64 head0, 64:128 head1
                        pe = qT[:, 2 * S:3 * S]
                        rsh = rshp.tile([128, S], BF16, tag="rsh")
                        nc.vector.stream_shuffle(
                            rsh[:, :].bitcast(F32), pe.bitcast(F32), SHUF_MASK)
                        nc.vector.tensor_tensor(out=rsh[:, :], in0=rsh[:, :],
                                                in1=ssq_t[:, :], op=OP.mult)
                        nc.vector.tensor_tensor(out=pe, in0=pe,
                                                in1=csq_t[:, :], op=OP.mult)
                        nc.vector.tensor_tensor(out=pe, in0=pe, in1=rsh[:, :],
                                                op=OP.add)
                        return qT

                    def emit_knv(g):
                        # k_nope^T (2 m-tiles) and v (token-major)
                        knT = gkvp.tile([128, 2 * S], BF16, tag="knT",
                                        name=f"knT_{g}")
                        for m in range(2):
                            wt = wkp.tile([128, 4, 128], BF16, tag="wk",
                                          name=f"wk_{g}_{m}")
                            nc.sync.dma_start(out=wt[:, :, :],
                                              in_=wk_d[:, g, m, :, :])
                            for qh in range(2):
                                ps = pjps.tile([128, 512], F32, tag="pj")
                                for k in range(4):
                                    nc.tensor.matmul(
                                        ps[:, :], wt[:, k, :],
                                        kv_cnT[:, k * S + qh * 512:
                                               k * S + qh * 512 + 512],
                                        start=(k == 0), stop=(k == 3))
                                nc.scalar.copy(knT[:, m * S + qh * 512:
                                                   m * S + qh * 512 + 512],
                                               ps[:, :])

                        v_sb = gkvp.tile([128, TOKT * G * VD], BF16, tag="v",
                                         name=f"v_{g}")
                        wv_t = wvp.tile([128, 4, 256], BF16, tag="wv",
                                        name=f"wv_{g}")
                        nc.sync.dma_start(out=wv_t[:, :, :], in_=wv_d[:, g, :, :])
                        for t in range(TOKT):
                            ps = pjps.tile([128, 512], F32, tag="pj")
                            for k in range(4):
                                nc.tensor.matmul(
                                    ps[:, :256],
                                    kv_cnT[:, k * S + t * 128:
                                           k * S + (t + 1) * 128],
                                    wv_t[:, k, :], start=(k == 0), stop=(k == 3))
                            nc.vector.tensor_scalar_mul(
                                v_sb[:, t * 256:(t + 1) * 256], ps[:, :256], 1.0)
                        return knT, v_sb

                    def wo_dma(hr):
                        wt = wop.tile([128, HPC, 128], BF16, tag="wo",
                                      name=f"wo_{hr}")
                        nc.sync.dma_start(out=wt[:, :, :], in_=wo_d[:, hr, :, :])
                        return wt

                    def kv_finalize():
                        # partner kv tiles + stats merge; rstd applied to
                        # all four slots (deadline: the knv matmuls)
                        nc.sync.dma_start(
                            out=kv_cnT[:, KVOWN * S:4 * S]
                                .rearrange("p (k t) -> p k t", k=2),
                            in_=gath_k[partner, 0:256, :]
                                .rearrange("(k p) t -> p k t", p=128))
                        nc.sync.dma_start(
                            out=prow_kv[:, :],
                            in_=gath_k[partner, 256:258, :].bitcast(F32))
                        for mj in range(KVOWN, 4):
                            dsl = kv_cnT[:, mj * S:(mj + 1) * S]
                            nc.vector.tensor_tensor(
                                out=dsl, in0=dsl, in1=mb_kv[:, :],
                                op=OP.subtract)
                        nc.vector.tensor_tensor(
                            out=sqrow_kv[:, :], in0=sqrow_kv[:, :],
                            in1=prow_kv[:, :], op=OP.add)
                        rstd_chain(vrow_kv, sqrow_kv, mrow_kv, rb_kv)
                        for mj in range(4):
                            dsl = kv_cnT[:, mj * S:(mj + 1) * S]
                            nc.vector.tensor_tensor(
                                out=dsl, in0=dsl, in1=rb_kv[:, :], op=OP.mult)

                    wq_t = {0: qT_dma(0)}
                    qts = {0: qT_mm(0, wq_t.pop(0))}
                    wq_t[1] = qT_dma(1)
                    qts[1] = qT_mm(1, wq_t.pop(1))
                    wq_t[2] = qT_dma(2)
                    kv_finalize()
                    knv = {}
                    for gg in range(3):
                        knv[gg] = emit_knv(gg)
                    wo_pre = []

                    for g in range(NG):
                        knT, v_sb = knv.pop(g)
                        qT = qts.pop(g)
                        if g == NG - 1:
                            wo_pre = [wo_dma(0), wo_dma(1)]

                        # ---- attention: heads interleaved per block,
                        # sum/pv pipelined one block behind the scores ----
                        for qh in range(2):
                            nik = 4 * (qh + 1)
                            po = [ops.tile([128, 512], F32, tag=f"po{hh}",
                                           name=f"po_{g}_{qh}_{hh}")
                                  for hh in range(G)]
                            psm = [smps.tile([1, 512], F32, tag=f"pm{hh}",
                                             name=f"pm_{g}_{qh}_{hh}")
                                   for hh in range(G)]

                            def scores(ik):
                                lo = max(128 * ik, 512 * qh)
                                hi = 512 * (qh + 1)
                                w = hi - lo
                                diag = (lo == 128 * ik)
                                p_l = []
                                for hh in range(G):
                                    ps_s = sps.tile([128, 512], F32,
                                                    tag=f"ps{hh}")
                                    nc.tensor.matmul(
                                        ps_s[:, :w],
                                        knT[:, hh * S + ik * 128:
                                            hh * S + (ik + 1) * 128],
                                        qT[:, hh * S + lo: hh * S + hi],
                                        start=True, stop=False)
                                    if diag:
                                        nc.tensor.matmul(
                                            ps_s[:, 0:128], id_t[:, :],
                                            mask_t[:, :],
                                            start=False, stop=False)
                                    nc.tensor.matmul(
                                        ps_s[:, :w],
                                        kpeT2[hh * 64:(hh + 1) * 64,
                                              ik * 128:(ik + 1) * 128],
                                        qT[hh * 64:(hh + 1) * 64,
                                           2 * S + lo: 2 * S + hi],
                                        start=False, stop=True)
                                    p = ppool.tile([128, 512], BF16,
                                                   tag=f"p{hh}")
                                    nc.scalar.activation(p[:, :w], ps_s[:, :w],
                                                         AF.Exp, scale=SCALE)
                                    p_l.append((p, w, lo))
                                return p_l

                            def sum_pv(ik, p_l):
                                for hh in range(G):
                                    p, w, lo = p_l[hh]
                                    osl = slice(lo - 512 * qh, hi_q - 512 * qh)
                                    nc.tensor.matmul(
                                        psm[hh][:, osl],
                                        ones_t[:, :], p[:, :w],
                                        start=(ik == 0), stop=(ik == nik - 1))
                                    nc.tensor.matmul(
                                        po[hh][:, osl],
                                        v_sb[:, ik * 256 + hh * 128:
                                             ik * 256 + (hh + 1) * 128],
                                        p[:, :w],
                                        start=(ik == 0), stop=(ik == nik - 1))

                            hi_q = 512 * (qh + 1)
                            prev = None
                            for ik in range(nik):
                                p_l = scores(ik)
                                if prev is not None:
                                    sum_pv(prev[0], prev[1])
                                prev = (ik, p_l)
                            sum_pv(prev[0], prev[1])

                            for hh in range(G):
                                hg = g * G + hh
                                srow = denp.tile([1, 512], F32, tag=f"dr{hh}")
                                nc.scalar.copy(srow[:, :], psm[hh][:, :])
                                rbc = denp.tile([128, 512], F32, tag=f"db{hh}")
                                nc.gpsimd.partition_broadcast(rbc[:, :],
                                                              srow[:, :])
                                nc.vector.reciprocal_approx_fast(
                                    out=rbc[:, :], in_=rbc[:, :])
                                nc.vector.tensor_tensor(
                                    out=oT[:, hg * S + qh * 512:
                                           hg * S + qh * 512 + 512],
                                    in0=po[hh][:, :], in1=rbc[:, :],
                                    op=OP.mult)

                        if g + 3 < NG:
                            knv[g + 3] = emit_knv(g + 3)
                        if g + 2 < NG:
                            qts[g + 2] = qT_mm(g + 2, wq_t.pop(g + 2))
                        if g + 3 < NG:
                            wq_t[g + 3] = qT_dma(g + 3)

                    # ======== phase 3: out^T = Wo^T @ o ========
                    for hr in range(KH):
                        wt = wo_pre[hr] if hr < len(wo_pre) else wo_dma(hr)
                        ot = outp.tile([128, S], F32, tag="out")
                        for qh in range(2):
                            sl = slice(qh * 512, qh * 512 + 512)
                            ps = sps.tile([128, 512], F32, tag=f"ps{qh}")
                            for m in range(HPC):
                                nc.tensor.matmul(
                                    ps[:, :], wt[:, m, :],
                                    oT[:, m * S + qh * 512:
                                       m * S + qh * 512 + 512],
                                    start=(m == 0), stop=(m == HPC - 1))
                            if qh == 0:
                                nc.scalar.copy(ot[:, sl], ps[:, :])
                            else:
                                nc.vector.tensor_scalar_mul(ot[:, sl], ps[:, :], 1.0)
                        nc.sync.dma_start(
                            out=outT[hr * 128:(hr + 1) * 128, :], in_=ot[:, :])
    nc.compile()
    return nc


def _host_prep(inputs):
    hs = np.asarray(inputs["hidden_states"], np.float32)
    cos = np.asarray(inputs["cos"], np.float32)
    sin = np.asarray(inputs["sin"], np.float32)
    pid = np.asarray(inputs["position_ids"]).astype(np.int64)
    Wqa = np.asarray(inputs["Wqa"], np.float32)
    gqa = np.asarray(inputs["gqa"], np.float32)
    Wqb = np.asarray(inputs["Wqb"], np.float32)
    Wkva = np.asarray(inputs["Wkva"], np.float32)
    gkva = np.asarray(inputs["gkva"], np.float32)
    Wkvb = np.asarray(inputs["Wkvb"], np.float32)
    Wo = np.asarray(inputs["Wo"], np.float32)

    # phase-1 fused projection: [pe'+sums | qa x6 | kv x4] m-tiles.
    # pe tile cols: 0:64 rope-permuted Wkva-pe, 64 sum(Wqa cols),
    # 96 sum(Wkva kv cols) — yields feature-sum rows for the LN means.
    wsum_qa = Wqa.sum(axis=1, keepdims=True)
    wsum_kv = Wkva[:, :KVL].sum(axis=1, keepdims=True)
    pe_cols = np.concatenate(
        [Wkva[:, KVL:][:, DIMS_PERM], wsum_qa, np.zeros((H, 31), np.float32),
         wsum_kv, np.zeros((H, 31), np.float32)], axis=1)
    # per-parity fused projection: shared [pe] + own qa half (6 tiles)
    # + own kv half (2 tiles)
    wa_t_par = []
    for t in range(TP):
        wa = np.concatenate(
            [pe_cols, Wqa[:, t * OWN * 128:(t + 1) * OWN * 128],
             Wkva[:, t * KVOWN * 128:KVL][:, :KVOWN * 128]], axis=1)
        wa_t_par.append(np.ascontiguousarray(
            wa.reshape(KH, 128, NMT, 128).transpose(1, 2, 0, 3)).astype(NPBF))

    # fold LN gains into the B-projections (bias terms are zero per spec)
    Wqb = Wqb * gqa[:, None]
    Wkvb = Wkvb * gkva[:, None]

    # sign pattern for the shuffle-based rotate_half
    sign = np.where(DIMS_PERM < RD // 2, -1.0, 1.0).astype(np.float32)[:, None]

    kp, q = np.mgrid[0:128, 0:128]
    maskL = np.where(q < kp, MASKV, 0.0).astype(np.float32)
    ident = np.eye(128, dtype=np.float32)

    per_core = []
    w4 = Wqb.reshape(QL, NH, QHD)
    wk4 = Wkvb.reshape(KVL, NH, ND + VD)
    for c in range(NCORES):
        b, t = divmod(c, TP)
        heads = slice(t * HPC, (t + 1) * HPC)
        # Wqb: group-blocked [h0 nope | h1 nope | h0 pe' | h1 pe'] per group
        wq = w4[:, heads]                       # (QL, 16, 192)
        nope = wq[:, :, :ND]                    # (QL, 16, 128)
        pe = wq[:, :, ND:][:, :, DIMS_PERM]     # (QL, 16, 64) permuted
        blocks = []
        for g in range(NG):
            blocks.extend([nope[:, 2 * g], nope[:, 2 * g + 1],
                           pe[:, 2 * g], pe[:, 2 * g + 1]])
        wqb_c = np.concatenate(blocks, axis=1)  # (QL, NG*384)
        # k-subtile order must match this core's q_anT slots: own half first
        kperm = list(range(t * OWN, t * OWN + OWN)) \
            + list(range((1 - t) * OWN, (1 - t) * OWN + OWN))
        # -> (128, NG, 3, 12, 128)
        wqb_t = np.ascontiguousarray(
            wqb_c.reshape(12, 128, NG, 3, 128)[kperm].transpose(1, 2, 3, 0, 4)
        ).astype(NPBF)

        # kv_cnT is own-tiles-first on each core; permute the contraction
        # axis of the decompression weights to match
        kperm_kv = [KVOWN * t, KVOWN * t + 1,
                    KVOWN * (1 - t), KVOWN * (1 - t) + 1]
        wkc = wk4[:, heads]
        wkvbk_c = wkc[:, :, :ND].reshape(KVL, HPC * ND)
        # -> (128, NG, 2, 4, 128)
        wk_t = np.ascontiguousarray(
            wkvbk_c.reshape(4, 128, NG, 2, 128)[kperm_kv]
            .transpose(1, 2, 3, 0, 4)
        ).astype(NPBF)
        wkvbv_c = wkc[:, :, ND:].reshape(KVL, HPC * VD)
        # -> (128, NG, 4, 256)
        wv_t = np.ascontiguousarray(
            wkvbv_c.reshape(4, 128, NG, 256)[kperm_kv].transpose(1, 2, 0, 3)
        ).astype(NPBF)

        wo_c = Wo[t * HPC * VD:(t + 1) * HPC * VD]   # (2048, 4096)
        # -> (128, 32, 16, 128)
        wo_t = np.ascontiguousarray(
            wo_c.reshape(HPC, 128, KH, 128).transpose(1, 2, 0, 3)).astype(NPBF)

        cos_g = cos[pid[b]]                     # (S, RD)
        sin_g = sin[pid[b]]
        cosT = cos_g.T[DIMS_PERM]               # (64, S)
        sinT = sin_g.T[DIMS_PERM]
        csq = np.ascontiguousarray(np.vstack([cosT, cosT])).astype(NPBF)
        ssq = np.ascontiguousarray(np.vstack([sinT * sign, sinT * sign])).astype(NPBF)

        hsT = hs[b].T                           # (H, S)
        hs_t = np.ascontiguousarray(
            hsT.reshape(KH, 128, S).transpose(1, 0, 2)).astype(NPBF)

        per_core.append({
            "hs_d": hs_t,
            "wa_d": wa_t_par[t],
            "wqb_d": wqb_t,
            "wk_d": wk_t,
            "wv_d": wv_t,
            "wo_d": wo_t,
            "csq_d": csq,
            "ssq_d": ssq,
            "mask_d": maskL.astype(NPBF),
            "id_d": ident.astype(NPBF),
            "ones_d": np.ones((128, 1), NPBF),
        })
    return per_core


def kernel(**inputs):
    if "nc" not in _NC_CACHE:
        _NC_CACHE["nc"] = _build_nc()
    nc = _NC_CACHE["nc"]
    in_maps = _host_prep(inputs)
    res = bass_utils.run_bass_kernel_spmd(nc, in_maps, core_ids=list(range(NCORES)))
    outs = []
    for b in range(B):
        acc = res.results[TP * b]["outT"].astype(np.float32)
        for t in range(1, TP):
            acc = acc + res.results[TP * b + t]["outT"]
        outs.append(acc.T)
    return np.stack(outs, axis=0)


# revision 39
# speedup vs baseline: 1.0407x; 1.0407x over previous
"""MLA (multi-headed latent attention) forward on 8 Trainium2 NeuronCores.

Sharding: data-parallel over batch (4) x tensor-parallel over heads (2):
core c handles batch c//2 with heads [16*(c%2), 16*(c%2)+16).
Each core computes a partial (H-dim) output contribution; host sums the
TP pair and stacks batches.

All matmuls run in bf16 (fp32 PSUM accumulation). Weights are host-
pre-tiled into the exact SBUF layouts so every DMA is contiguous.
LN means are folded into the fused projection as extra weight columns.

Schedule (the point of this structure is to keep the PE gap-free so the
HAM clock gate stays at 2.4 GHz):
- phase-1 m-tile order pe, qa x6, kv x4 so both TP AllGathers launch
  early and complete while the PE chews the kv tiles;
- mean subtraction happens on the raw bf16 projections as soon as the
  inputs exist; the qa rstd is folded into the qb-projection copies so
  the sqrt chain is never on the PE critical path;
- the causal mask is a third accumulating matmul on diagonal score
  blocks (exp of -30000 -> 0), removing the vector hop from the
  softmax chain: PE -> Act -> PE;
- the two heads of each group are interleaved block-by-block and
  sum/pv matmuls are software-pipelined one block behind the scores,
  so there is always independent PE work while Act computes exp.
Layout is feature-major ("T" = [feature, token]) everywhere except v
(token-major for the PV matmul).
"""

import numpy as np
import ml_dtypes
import concourse.bass as bass
import concourse.mybir as mybir
import concourse.tile as tile
from concourse import bacc
from concourse import bass_utils

F32 = mybir.dt.float32
BF16 = mybir.dt.bfloat16
AX = mybir.AxisListType
OP = mybir.AluOpType
AF = mybir.ActivationFunctionType
NPBF = ml_dtypes.bfloat16

B, S, H, NH = 4, 1024, 4096, 32
QL, KVL, RD, ND, VD = 1536, 512, 64, 128, 128
QHD = ND + RD  # 192
EPS = 1e-6
NCORES = 8
TP = 2                 # tensor-parallel ways (heads)
HPC = NH // TP         # 16 heads per core
G = 2                  # heads per group
NG = HPC // G          # 8 groups
TOKT = S // 128        # 8 token tiles
KH = H // 128          # 32 contraction tiles for H
NMT = 9                # phase-1 m-tiles/core: 1 pe+stats, 6 own qa, 2 own kv
OWN = 6                # qa m-tiles computed locally; partner half via AllGather
KVOWN = 2              # kv m-tiles computed locally; partner half via AllGather
CC_GROUPS = [[0, 1], [2, 3], [4, 5], [6, 7]]
SCALE = float(QHD) ** -0.5
MASKV = -30000.0

# rope feature permutation: pairs (d, d+32) land 16 lanes apart within a
# 32-partition quadrant so stream_shuffle can do rotate_half.
DIMS_PERM = np.array(
    list(range(0, 16)) + list(range(32, 48))
    + list(range(16, 32)) + list(range(48, 64)), dtype=np.int64)
SHUF_MASK = [(i + 16) % 32 for i in range(32)]

_NC_CACHE = {}


def _build_nc():
    nc = bacc.Bacc("TRN2", target_bir_lowering=False, debug=False)

    hs_d = nc.dram_tensor("hs_d", (128, KH, S), BF16, kind="ExternalInput").ap()
    wa_d = nc.dram_tensor("wa_d", (128, NMT, KH, 128), BF16, kind="ExternalInput").ap()
    wqb_d = nc.dram_tensor("wqb_d", (128, NG, 3, 12, 128), BF16, kind="ExternalInput").ap()
    wk_d = nc.dram_tensor("wk_d", (128, NG, 2, 4, 128), BF16, kind="ExternalInput").ap()
    wv_d = nc.dram_tensor("wv_d", (128, NG, 4, 256), BF16, kind="ExternalInput").ap()
    wo_d = nc.dram_tensor("wo_d", (128, KH, HPC, 128), BF16, kind="ExternalInput").ap()
    csq_d = nc.dram_tensor("csq_d", (128, S), BF16, kind="ExternalInput").ap()
    ssq_d = nc.dram_tensor("ssq_d", (128, S), BF16, kind="ExternalInput").ap()
    mask_d = nc.dram_tensor("mask_d", (128, 128), BF16, kind="ExternalInput").ap()
    id_d = nc.dram_tensor("id_d", (128, 128), BF16, kind="ExternalInput").ap()
    ones_d = nc.dram_tensor("ones_d", (128, 1), BF16, kind="ExternalInput").ap()
    outT = nc.dram_tensor("outT", (H, S), F32, kind="ExternalOutput").ap()
    stage_a = nc.dram_tensor("stage_a", (512, S), BF16, kind="Internal").ap()
    stage_b = nc.dram_tensor("stage_b", (258, S), BF16, kind="Internal").ap()
    stage_k = nc.dram_tensor("stage_k", (258, S), BF16, kind="Internal").ap()
    gath_a = nc.dram_tensor("gath_a", (2, 512, S), BF16, kind="Internal").ap()
    gath_b = nc.dram_tensor("gath_b", (2, 258, S), BF16, kind="Internal").ap()
    gath_k = nc.dram_tensor("gath_k", (2, 258, S), BF16, kind="Internal").ap()

    with tile.TileContext(nc) as tc:
        with tc.tile_pool(name="pers", bufs=1) as pers:
            # ---------------- persistent tiles ----------------
            q_anT = pers.tile([128, 12 * S], BF16)     # (q_a - m)^T  (1536, 1024)
            kv_cnT = pers.tile([128, 4 * S], BF16)     # LN(kv_c)^T (512, 1024)
            kpeT2 = pers.tile([128, S], BF16)          # roped k_pe^T, both halves
            csq_t = pers.tile([128, S], BF16)
            ssq_t = pers.tile([128, S], BF16)
            mask_t = pers.tile([128, 128], BF16)
            id_t = pers.tile([128, 128], BF16)
            ones_t = pers.tile([128, 1], BF16)
            # LN stat rows: all at partition 0 (engine base-partition rules
            # and the Q7 broadcast assume it); the prow tiles double as the
            # rstd-chain scratch once their add has consumed them.
            mrow_qa = pers.tile([1, S], F32, name="mrow_qa")
            mrow_kv = pers.tile([1, S], F32, name="mrow_kv")
            sqrow_qa = pers.tile([1, S], F32, name="sqrow_qa")
            sqrow_kv = pers.tile([1, S], F32, name="sqrow_kv")
            prow_qa = pers.tile([1, S], F32, name="prow_qa")
            prow_kv = pers.tile([1, S], F32, name="prow_kv")
            mb_qa = pers.tile([128, S], BF16, name="mb_qa")    # mean bcast
            mb_kv = pers.tile([128, S], BF16, name="mb_kv")
            rb_qa = pers.tile([128, S], F32, name="rb_qa")     # rstd bcast
            rb_kv = pers.tile([128, S], F32, name="rb_kv")

            # ones first (tiny, needed by the stats matmuls); mask/id via
            # the software-DGE pool queue (needed only by attention).
            nc.scalar.dma_start(out=ones_t[:, :], in_=ones_d)
            nc.gpsimd.dma_start(out=mask_t[:, :], in_=mask_d)
            nc.gpsimd.dma_start(out=id_t[:, :], in_=id_d)

            def rstd_chain(v1, sqrow, mrow, rb):
                # rows ([1,S]): var = E[x^2] - m^2; rstd = 1/sqrt(var+eps)
                nc.vector.tensor_tensor(out=v1[:, :], in0=mrow[:, :],
                                        in1=mrow[:, :], op=OP.mult)
                nc.vector.tensor_tensor(out=v1[:, :], in0=sqrow[:, :],
                                        in1=v1[:, :], op=OP.subtract)
                nc.vector.tensor_scalar_add(v1[:, :], v1[:, :], EPS)
                nc.scalar.sqrt(v1[:, :], v1[:, :])
                nc.vector.reciprocal_approx_fast(out=v1[:, :], in_=v1[:, :])
                nc.gpsimd.partition_broadcast(rb[:, :], v1[:, :])

            # ======== phase 1 (feature-major): X^T = Wa^T @ hs^T ====
            # m-tile order: pe+stats first (means), qa0-3 (first gather),
            # then the own kv tiles (kv gather hides under qa4/qa5), then
            # qa4/qa5 (second gather hides under the phase-2 qb work).
            M_TILES = ([("pe", 0)] + [("qa", i) for i in range(4)]
                       + [("kv", i) for i in range(KVOWN)]
                       + [("qa", i) for i in range(4, OWN)])

            with tc.tile_pool(name="hsp", bufs=1) as hsp, \
                 tc.tile_pool(name="p1wa", bufs=3) as p1wa, \
                 tc.tile_pool(name="sqp", bufs=2) as sqp, \
                 tc.tile_pool(name="rowp", bufs=1) as rowp, \
                 tc.tile_pool(name="p1ps", bufs=3, space="PSUM") as p1ps, \
                 tc.tile_pool(name="stps", bufs=1, space="PSUM") as stps:
                hst = [hsp.tile([128, 4, S], BF16, name=f"hst_{i}")
                       for i in range(8)]
                # evens on the Act queue, odds interleaved between the early
                # weight tiles on the SP queue; cos/sin trail the Act stream.
                for i in (0, 2, 4, 6):
                    nc.scalar.dma_start(out=hst[i][:, :, :],
                                        in_=hs_d[:, 4 * i:4 * (i + 1), :])
                nc.scalar.dma_start(out=csq_t[:, :], in_=csq_d)
                nc.scalar.dma_start(out=ssq_t[:, :], in_=ssq_d)

                # sum-of-squares stats accumulators (fp32 PSUM)
                stat = {("qa", 0): stps.tile([1, 512], F32, name="st_qa0"),
                        ("qa", 1): stps.tile([1, 512], F32, name="st_qa1"),
                        ("kv", 0): stps.tile([1, 512], F32, name="st_kv0"),
                        ("kv", 1): stps.tile([1, 512], F32, name="st_kv1")}

                def mean_bcast(kind, mrow, mb):
                    # f32 mean row -> bf16 row -> [128,S] broadcast
                    r16 = rowp.tile([1, S], BF16, name=f"r16_{kind}")
                    nc.vector.tensor_scalar_mul(r16[:, :], mrow[:, :], 1.0)
                    nc.gpsimd.partition_broadcast(mb[:, :], r16[:, :])

                def tile_epilogue(kind, mi, dest):
                    # Square + stats matmuls + staging for one m-tile.
                    # Deferred one m-tile so the in-order PE queue never
                    # waits on the Act-queue Square before the next chains.
                    nmt = OWN if kind == "qa" else KVOWN
                    sqt = sqp.tile([128, S], BF16, tag="sq")
                    nc.scalar.activation(sqt[:, :], dest, AF.Square)
                    for qh in range(2):
                        sl = slice(qh * 512, qh * 512 + 512)
                        nc.tensor.matmul(
                            stat[(kind, qh)][:, :], ones_t[:, :], sqt[:, sl],
                            start=(mi == 0), stop=(mi == nmt - 1))
                    if kind == "qa":
                        # stage raw tiles for the TP-pair exchange
                        stg = stage_a if mi < 4 else stage_b
                        ro = (mi if mi < 4 else mi - 4) * 128
                        nc.sync.dma_start(out=stg[ro:ro + 128, :], in_=dest)
                        if mi == 3:
                            nc.gpsimd.collective_compute(
                                "AllGather", OP.bypass,
                                replica_groups=CC_GROUPS,
                                ins=[stage_a], outs=[gath_a])
                        if mi == OWN - 1:
                            # own E[x^2] rows, staged bitcast + second gather
                            for qh in range(2):
                                sl = slice(qh * 512, qh * 512 + 512)
                                nc.vector.tensor_scalar_mul(
                                    sqrow_qa[:, sl], stat[("qa", qh)][:, :],
                                    1.0 / QL)
                            sq16 = sqrow_qa[:, :].bitcast(BF16)  # [1, 2048]
                            nc.sync.dma_start(out=stage_b[256:257, :],
                                              in_=sq16[:, 0:S])
                            nc.sync.dma_start(out=stage_b[257:258, :],
                                              in_=sq16[:, S:2 * S])
                            nc.gpsimd.collective_compute(
                                "AllGather", OP.bypass,
                                replica_groups=CC_GROUPS,
                                ins=[stage_b], outs=[gath_b])
                            # mean-subtract the own qa slices now (rstd is
                            # folded into the qb copies later)
                            for mj in range(OWN):
                                dsl = q_anT[:, mj * S:(mj + 1) * S]
                                nc.vector.tensor_tensor(
                                    out=dsl, in0=dsl, in1=mb_qa[:, :],
                                    op=OP.subtract)
                    else:  # kv: stage for exchange, mean-subtract in place
                        nc.sync.dma_start(
                            out=stage_k[mi * 128:(mi + 1) * 128, :], in_=dest)
                        if mi == KVOWN - 1:
                            for qh in range(2):
                                sl = slice(qh * 512, qh * 512 + 512)
                                nc.vector.tensor_scalar_mul(
                                    sqrow_kv[:, sl], stat[("kv", qh)][:, :],
                                    1.0 / KVL)
                            sqk16 = sqrow_kv[:, :].bitcast(BF16)
                            nc.sync.dma_start(out=stage_k[256:257, :],
                                              in_=sqk16[:, 0:S])
                            nc.sync.dma_start(out=stage_k[257:258, :],
                                              in_=sqk16[:, S:2 * S])
                            nc.gpsimd.collective_compute(
                                "AllGather", OP.bypass,
                                replica_groups=CC_GROUPS,
                                ins=[stage_k], outs=[gath_k])
                        nc.vector.tensor_tensor(out=dest, in0=dest,
                                                in1=mb_kv[:, :], op=OP.subtract)

                first_sync_hs = [False]
                pending = None

                for (kind, mi) in M_TILES:
                    tix = {"pe": 0, "qa": 1, "kv": 7}[kind] + mi
                    wt = p1wa.tile([128, KH, 128], BF16, tag="wa",
                                   name=f"wa_{kind}_{mi}")
                    nc.sync.dma_start(out=wt[:, :, :], in_=wa_d[:, tix, :, :])
                    if not first_sync_hs[0]:
                        # odd hs tiles follow the first weight tile on sync;
                        # they must all be emitted before the first m-tile's
                        # matmuls (deps follow emission order)
                        first_sync_hs[0] = True
                        for i in (1, 3, 5, 7):
                            nc.sync.dma_start(
                                out=hst[i][:, :, :],
                                in_=hs_d[:, 4 * i:4 * (i + 1), :])
                    if kind == "qa":
                        dest = q_anT[:, mi * S:(mi + 1) * S]
                    elif kind == "kv":
                        dest = kv_cnT[:, mi * S:(mi + 1) * S]
                    else:
                        dest = kpeT2[0:64, :]
                    for qh in range(2):
                        sl = slice(qh * 512, qh * 512 + 512)
                        ps = p1ps.tile([128, 512], F32, tag="p1")
                        for k in range(KH):
                            nc.tensor.matmul(
                                ps[:, :], wt[:, k, :], hst[k // 4][:, k % 4, sl],
                                start=(k == 0), stop=(k == KH - 1))
                        if kind == "pe":
                            nc.scalar.copy(dest[:, sl], ps[0:64, :])
                            # rows 64/96 carry sum(qa), sum(kv) over features
                            nc.vector.tensor_scalar_mul(
                                mrow_qa[:, sl], ps[64:65, :], 1.0 / QL)
                            nc.vector.tensor_scalar_mul(
                                mrow_kv[:, sl], ps[96:97, :], 1.0 / KVL)
                        else:
                            nc.scalar.copy(dest[:, sl], ps[:, :])
                    if kind == "pe":
                        # means -> bf16 broadcasts (early: deps ready now)
                        mean_bcast("qa", mrow_qa, mb_qa)
                        mean_bcast("kv", mrow_kv, mb_kv)
                        # rope k_pe rows 0:64, duplicate into 64:128 (the
                        # duplicate is a software-DGE copy; slack is huge)
                        kp_sh = sqp.tile([64, S], BF16, tag="kpsh", name="kpsh")
                        nc.vector.stream_shuffle(
                            kp_sh[:, :].bitcast(F32), kpeT2[0:64, :].bitcast(F32),
                            SHUF_MASK)
                        nc.vector.tensor_tensor(out=kp_sh[:, :], in0=kp_sh[:, :],
                                                in1=ssq_t[:64, :], op=OP.mult)
                        nc.vector.tensor_tensor(out=kpeT2[0:64, :],
                                                in0=kpeT2[0:64, :],
                                                in1=csq_t[:64, :], op=OP.mult)
                        nc.vector.tensor_tensor(out=kpeT2[0:64, :],
                                                in0=kpeT2[0:64, :],
                                                in1=kp_sh[:, :], op=OP.add)
                        nc.gpsimd.dma_start(out=kpeT2[64:128, :],
                                            in_=kpeT2[0:64, :])
                        continue
                    if pending is not None:
                        tile_epilogue(*pending)
                    if kind == "kv" and mi == KVOWN - 1:
                        # flush inline: the kv gather must launch now so the
                        # exchange hides under the remaining qa tiles
                        tile_epilogue(kind, mi, dest)
                        pending = None
                    else:
                        pending = (kind, mi, dest)
                if pending is not None:
                    tile_epilogue(*pending)

                pid = nc.sync.partition_id()
                partner = 1 - (pid % 2)

            # ======== phase 2 + 3 ========
            with tc.tile_pool(name="otp", bufs=1) as otp:
                oT = otp.tile([128, HPC * S], BF16)    # normalized o^T
                with tc.tile_pool(name="gq2", bufs=3) as gqp, \
                     tc.tile_pool(name="wqp", bufs=2) as wqp, \
                     tc.tile_pool(name="wop", bufs=2) as wop, \
                     tc.tile_pool(name="op", bufs=2) as outp, \
                     tc.tile_pool(name="gkv", bufs=3) as gkvp, \
                     tc.tile_pool(name="wk", bufs=3) as wkp, \
                     tc.tile_pool(name="wv", bufs=2) as wvp, \
                     tc.tile_pool(name="rshp", bufs=1) as rshp, \
                     tc.tile_pool(name="pp", bufs=2) as ppool, \
                     tc.tile_pool(name="denp", bufs=1) as denp, \
                     tc.tile_pool(name="pjps", bufs=2, space="PSUM") as pjps, \
                     tc.tile_pool(name="sps", bufs=1, space="PSUM") as sps, \
                     tc.tile_pool(name="ops", bufs=1, space="PSUM") as ops, \
                     tc.tile_pool(name="smps", bufs=1, space="PSUM") as smps:

                    def qT_dma(g):
                        wt = wqp.tile([128, 3, 12, 128], BF16, tag="wqb",
                                      name=f"wqb_{g}")
                        nc.sync.dma_start(out=wt[:, :, :, :],
                                          in_=wqb_d[:, g, :, :, :])
                        return wt

                    def qT_mm(g, wt):
                        # q^T for this group: 2 nope m-tiles + 1 pe pair;
                        # the qa rstd is folded into the PSUM->SBUF copies.
                        qT = gqp.tile([128, 3 * S], BF16, tag="qT",
                                      name=f"qT_{g}")
                        for m in range(3):
                            for qh in range(2):
                                sl = slice(qh * 512, qh * 512 + 512)
                                ps = pjps.tile([128, 512], F32, tag="pj")
                                for k in range(12):
                                    nc.tensor.matmul(
                                        ps[:, :], wt[:, m, k, :],
                                        q_anT[:, k * S + qh * 512:
                                              k * S + qh * 512 + 512],
                                        start=(k == 0), stop=(k == 11))
                                nc.vector.tensor_tensor(
                                    out=qT[:, m * S + qh * 512:
                                           m * S + qh * 512 + 512],
                                    in0=ps[:, :], in1=rb_qa[:, sl], op=OP.mult)
                        # rope the pe tile (m=2): rows 0:64 head0, 64:128 head1
                        pe = qT[:, 2 * S:3 * S]
                        rsh = rshp.tile([128, S], BF16, tag="rsh")
                        nc.vector.stream_shuffle(
                            rsh[:, :].bitcast(F32), pe.bitcast(F32), SHUF_MASK)
                        nc.vector.tensor_tensor(out=rsh[:, :], in0=rsh[:, :],
                                                in1=ssq_t[:, :], op=OP.mult)
                        nc.vector.tensor_tensor(out=pe, in0=pe,
                                                in1=csq_t[:, :], op=OP.mult)
                        nc.vector.tensor_tensor(out=pe, in0=pe, in1=rsh[:, :],
                                                op=OP.add)
                        return qT

                    def emit_knv(g):
                        # k_nope^T (2 m-tiles) and v (token-major)
                        knT = gkvp.tile([128, 2 * S], BF16, tag="knT",
                                        name=f"knT_{g}")
                        for m in range(2):
                            wt = wkp.tile([128, 4, 128], BF16, tag="wk",
                                          name=f"wk_{g}_{m}")
                            nc.sync.dma_start(out=wt[:, :, :],
                                              in_=wk_d[:, g, m, :, :])
                            for qh in range(2):
                                ps = pjps.tile([128, 512], F32, tag="pj")
                                for k in range(4):
                                    nc.tensor.matmul(
                                        ps[:, :], wt[:, k, :],
                                        kv_cnT[:, k * S + qh * 512:
                                               k * S + qh * 512 + 512],
                                        start=(k == 0), stop=(k == 3))
                                nc.scalar.copy(knT[:, m * S + qh * 512:
                                                   m * S + qh * 512 + 512],
                                               ps[:, :])

                        v_sb = gkvp.tile([128, TOKT * G * VD], BF16, tag="v",
                                         name=f"v_{g}")
                        wv_t = wvp.tile([128, 4, 256], BF16, tag="wv",
                                        name=f"wv_{g}")
                        nc.sync.dma_start(out=wv_t[:, :, :], in_=wv_d[:, g, :, :])
                        for t in range(TOKT):
                            ps = pjps.tile([128, 512], F32, tag="pj")
                            for k in range(4):
                                nc.tensor.matmul(
                                    ps[:, :256],
                                    kv_cnT[:, k * S + t * 128:
                                           k * S + (t + 1) * 128],
                                    wv_t[:, k, :], start=(k == 0), stop=(k == 3))
                            nc.vector.tensor_scalar_mul(
                                v_sb[:, t * 256:(t + 1) * 256], ps[:, :256], 1.0)
                        return knT, v_sb

                    def wo_dma(hr):
                        wt = wop.tile([128, HPC, 128], BF16, tag="wo",
                                      name=f"wo_{hr}")
                        nc.sync.dma_start(out=wt[:, :, :], in_=wo_d[:, hr, :, :])
                        return wt

                    def qa_finalize():
                        # partner qa tiles + stats merge (waits on the
                        # second gather; hidden under the knv matmuls)
                        nc.sync.dma_start(
                            out=q_anT[:, OWN * S:10 * S]
                                .rearrange("p (k t) -> p k t", k=4),
                            in_=gath_a[partner, 0:512, :]
                                .rearrange("(k p) t -> p k t", p=128))
                        nc.sync.dma_start(
                            out=q_anT[:, 10 * S:12 * S]
                                .rearrange("p (k t) -> p k t", k=2),
                            in_=gath_b[partner, 0:256, :]
                                .rearrange("(k p) t -> p k t", p=128))
                        nc.sync.dma_start(
                            out=prow_qa[:, :],
                            in_=gath_b[partner, 256:258, :].bitcast(F32))
                        for mj in range(OWN, 12):
                            dsl = q_anT[:, mj * S:(mj + 1) * S]
                            nc.vector.tensor_tensor(
                                out=dsl, in0=dsl, in1=mb_qa[:, :],
                                op=OP.subtract)
                        nc.vector.tensor_tensor(
                            out=sqrow_qa[:, :], in0=sqrow_qa[:, :],
                            in1=prow_qa[:, :], op=OP.add)
                        rstd_chain(prow_qa, sqrow_qa, mrow_qa, rb_qa)

                    def kv_finalize():
                        # partner kv tiles + stats merge; rstd applied to
                        # all four slots (deadline: the knv matmuls)
                        nc.sync.dma_start(
                            out=kv_cnT[:, KVOWN * S:4 * S]
                                .rearrange("p (k t) -> p k t", k=2),
                            in_=gath_k[partner, 0:256, :]
                                .rearrange("(k p) t -> p k t", p=128))
                        nc.sync.dma_start(
                            out=prow_kv[:, :],
                            in_=gath_k[partner, 256:258, :].bitcast(F32))
                        for mj in range(KVOWN, 4):
                            dsl = kv_cnT[:, mj * S:(mj + 1) * S]
                            nc.vector.tensor_tensor(
                                out=dsl, in0=dsl, in1=mb_kv[:, :],
                                op=OP.subtract)
                        nc.vector.tensor_tensor(
                            out=sqrow_kv[:, :], in0=sqrow_kv[:, :],
                            in1=prow_kv[:, :], op=OP.add)
                        rstd_chain(prow_kv, sqrow_kv, mrow_kv, rb_kv)
                        for mj in range(4):
                            dsl = kv_cnT[:, mj * S:(mj + 1) * S]
                            nc.vector.tensor_tensor(
                                out=dsl, in0=dsl, in1=rb_kv[:, :], op=OP.mult)

                    # order: knv first (needs only the early kv exchange) so
                    # the PE chews decompression matmuls while the second qa
                    # gather completes; qT projections follow.
                    wq_t = {0: qT_dma(0), 1: qT_dma(1)}
                    kv_finalize()
                    knv = {}
                    for gg in range(3):
                        knv[gg] = emit_knv(gg)
                    qa_finalize()
                    qts = {0: qT_mm(0, wq_t.pop(0))}
                    qts[1] = qT_mm(1, wq_t.pop(1))
                    wq_t[2] = qT_dma(2)
                    wo_pre = []

                    for g in range(NG):
                        knT, v_sb = knv.pop(g)
                        qT = qts.pop(g)
                        if g == NG - 1:
                            wo_pre = [wo_dma(0), wo_dma(1)]

                        # ---- attention: heads interleaved per block,
                        # sum/pv pipelined one block behind the scores ----
                        for qh in range(2):
                            nik = 4 * (qh + 1)
                            po = [ops.tile([128, 512], F32, tag=f"po{hh}",
                                           name=f"po_{g}_{qh}_{hh}")
                                  for hh in range(G)]
                            psm = [smps.tile([1, 512], F32, tag=f"pm{hh}",
                                             name=f"pm_{g}_{qh}_{hh}")
                                   for hh in range(G)]

                            def scores(ik):
                                lo = max(128 * ik, 512 * qh)
                                hi = 512 * (qh + 1)
                                w = hi - lo
                                diag = (lo == 128 * ik)
                                p_l = []
                                for hh in range(G):
                                    ps_s = sps.tile([128, 512], F32,
                                                    tag=f"ps{hh}")
                                    nc.tensor.matmul(
                                        ps_s[:, :w],
                                        knT[:, hh * S + ik * 128:
                                            hh * S + (ik + 1) * 128],
                                        qT[:, hh * S + lo: hh * S + hi],
                                        start=True, stop=False)
                                    if diag:
                                        nc.tensor.matmul(
                                            ps_s[:, 0:128], id_t[:, :],
                                            mask_t[:, :],
                                            start=False, stop=False)
                                    nc.tensor.matmul(
                                        ps_s[:, :w],
                                        kpeT2[hh * 64:(hh + 1) * 64,
                                              ik * 128:(ik + 1) * 128],
                                        qT[hh * 64:(hh + 1) * 64,
                                           2 * S + lo: 2 * S + hi],
                                        start=False, stop=True)
                                    p = ppool.tile([128, 512], BF16,
                                                   tag=f"p{hh}")
                                    nc.scalar.activation(p[:, :w], ps_s[:, :w],
                                                         AF.Exp, scale=SCALE)
                                    p_l.append((p, w, lo))
                                return p_l

                            def sum_pv(ik, p_l):
                                for hh in range(G):
                                    p, w, lo = p_l[hh]
                                    osl = slice(lo - 512 * qh, hi_q - 512 * qh)
                                    nc.tensor.matmul(
                                        psm[hh][:, osl],
                                        ones_t[:, :], p[:, :w],
                                        start=(ik == 0), stop=(ik == nik - 1))
                                    nc.tensor.matmul(
                                        po[hh][:, osl],
                                        v_sb[:, ik * 256 + hh * 128:
                                             ik * 256 + (hh + 1) * 128],
                                        p[:, :w],
                                        start=(ik == 0), stop=(ik == nik - 1))

                            hi_q = 512 * (qh + 1)
                            prev = None
                            for ik in range(nik):
                                p_l = scores(ik)
                                if prev is not None:
                                    sum_pv(prev[0], prev[1])
                                prev = (ik, p_l)
                            sum_pv(prev[0], prev[1])

                            for hh in range(G):
                                hg = g * G + hh
                                srow = denp.tile([1, 512], F32, tag=f"dr{hh}")
                                nc.scalar.copy(srow[:, :], psm[hh][:, :])
                                rbc = denp.tile([128, 512], F32, tag=f"db{hh}")
                                nc.gpsimd.partition_broadcast(rbc[:, :],
                                                              srow[:, :])
                                nc.vector.reciprocal_approx_fast(
                                    out=rbc[:, :], in_=rbc[:, :])
                                nc.vector.tensor_tensor(
                                    out=oT[:, hg * S + qh * 512:
                                           hg * S + qh * 512 + 512],
                                    in0=po[hh][:, :], in1=rbc[:, :],
                                    op=OP.mult)

                        if g + 3 < NG:
                            knv[g + 3] = emit_knv(g + 3)
                        if g + 2 < NG:
                            qts[g + 2] = qT_mm(g + 2, wq_t.pop(g + 2))
                        if g + 3 < NG:
                            wq_t[g + 3] = qT_dma(g + 3)

                    # ======== phase 3: out^T = Wo^T @ o ========
                    for hr in range(KH):
                        wt = wo_pre[hr] if hr < len(wo_pre) else wo_dma(hr)
                        ot = outp.tile([128, S], F32, tag="out")
                        for qh in range(2):
                            sl = slice(qh * 512, qh * 512 + 512)
                            ps = sps.tile([128, 512], F32, tag=f"ps{qh}")
                            for m in range(HPC):
                                nc.tensor.matmul(
                                    ps[:, :], wt[:, m, :],
                                    oT[:, m * S + qh * 512:
                                       m * S + qh * 512 + 512],
                                    start=(m == 0), stop=(m == HPC - 1))
                            if qh == 0:
                                nc.scalar.copy(ot[:, sl], ps[:, :])
                            else:
                                nc.vector.tensor_scalar_mul(ot[:, sl], ps[:, :], 1.0)
                        nc.sync.dma_start(
                            out=outT[hr * 128:(hr + 1) * 128, :], in_=ot[:, :])
    nc.compile()
    return nc


def _host_prep(inputs):
    hs = np.asarray(inputs["hidden_states"], np.float32)
    cos = np.asarray(inputs["cos"], np.float32)
    sin = np.asarray(inputs["sin"], np.float32)
    pid = np.asarray(inputs["position_ids"]).astype(np.int64)
    Wqa = np.asarray(inputs["Wqa"], np.float32)
    gqa = np.asarray(inputs["gqa"], np.float32)
    Wqb = np.asarray(inputs["Wqb"], np.float32)
    Wkva = np.asarray(inputs["Wkva"], np.float32)
    gkva = np.asarray(inputs["gkva"], np.float32)
    Wkvb = np.asarray(inputs["Wkvb"], np.float32)
    Wo = np.asarray(inputs["Wo"], np.float32)

    # phase-1 fused projection: [pe'+sums | qa x6 | kv x4] m-tiles.
    # pe tile cols: 0:64 rope-permuted Wkva-pe, 64 sum(Wqa cols),
    # 96 sum(Wkva kv cols) — yields feature-sum rows for the LN means.
    wsum_qa = Wqa.sum(axis=1, keepdims=True)
    wsum_kv = Wkva[:, :KVL].sum(axis=1, keepdims=True)
    pe_cols = np.concatenate(
        [Wkva[:, KVL:][:, DIMS_PERM], wsum_qa, np.zeros((H, 31), np.float32),
         wsum_kv, np.zeros((H, 31), np.float32)], axis=1)
    # per-parity fused projection: shared [pe] + own qa half (6 tiles)
    # + own kv half (2 tiles)
    wa_t_par = []
    for t in range(TP):
        wa = np.concatenate(
            [pe_cols, Wqa[:, t * OWN * 128:(t + 1) * OWN * 128],
             Wkva[:, t * KVOWN * 128:KVL][:, :KVOWN * 128]], axis=1)
        wa_t_par.append(np.ascontiguousarray(
            wa.reshape(KH, 128, NMT, 128).transpose(1, 2, 0, 3)).astype(NPBF))

    # fold LN gains into the B-projections (bias terms are zero per spec)
    Wqb = Wqb * gqa[:, None]
    Wkvb = Wkvb * gkva[:, None]

    # sign pattern for the shuffle-based rotate_half
    sign = np.where(DIMS_PERM < RD // 2, -1.0, 1.0).astype(np.float32)[:, None]

    kp, q = np.mgrid[0:128, 0:128]
    maskL = np.where(q < kp, MASKV, 0.0).astype(np.float32)
    ident = np.eye(128, dtype=np.float32)

    per_core = []
    w4 = Wqb.reshape(QL, NH, QHD)
    wk4 = Wkvb.reshape(KVL, NH, ND + VD)
    for c in range(NCORES):
        b, t = divmod(c, TP)
        heads = slice(t * HPC, (t + 1) * HPC)
        # Wqb: group-blocked [h0 nope | h1 nope | h0 pe' | h1 pe'] per group
        wq = w4[:, heads]                       # (QL, 16, 192)
        nope = wq[:, :, :ND]                    # (QL, 16, 128)
        pe = wq[:, :, ND:][:, :, DIMS_PERM]     # (QL, 16, 64) permuted
        blocks = []
        for g in range(NG):
            blocks.extend([nope[:, 2 * g], nope[:, 2 * g + 1],
                           pe[:, 2 * g], pe[:, 2 * g + 1]])
        wqb_c = np.concatenate(blocks, axis=1)  # (QL, NG*384)
        # k-subtile order must match this core's q_anT slots: own half first
        kperm = list(range(t * OWN, t * OWN + OWN)) \
            + list(range((1 - t) * OWN, (1 - t) * OWN + OWN))
        # -> (128, NG, 3, 12, 128)
        wqb_t = np.ascontiguousarray(
            wqb_c.reshape(12, 128, NG, 3, 128)[kperm].transpose(1, 2, 3, 0, 4)
        ).astype(NPBF)

        # kv_cnT is own-tiles-first on each core; permute the contraction
        # axis of the decompression weights to match
        kperm_kv = [KVOWN * t, KVOWN * t + 1,
                    KVOWN * (1 - t), KVOWN * (1 - t) + 1]
        wkc = wk4[:, heads]
        wkvbk_c = wkc[:, :, :ND].reshape(KVL, HPC * ND)
        # -> (128, NG, 2, 4, 128)
        wk_t = np.ascontiguousarray(
            wkvbk_c.reshape(4, 128, NG, 2, 128)[kperm_kv]
            .transpose(1, 2, 3, 0, 4)
        ).astype(NPBF)
        wkvbv_c = wkc[:, :, ND:].reshape(KVL, HPC * VD)
        # -> (128, NG, 4, 256)
        wv_t = np.ascontiguousarray(
            wkvbv_c.reshape(4, 128, NG, 256)[kperm_kv].transpose(1, 2, 0, 3)
        ).astype(NPBF)

        wo_c = Wo[t * HPC * VD:(t + 1) * HPC * VD]   # (2048, 4096)
        # -> (128, 32, 16, 128)
        wo_t = np.ascontiguousarray(
            wo_c.reshape(HPC, 128, KH, 128).transpose(1, 2, 0, 3)).astype(NPBF)

        cos_g = cos[pid[b]]                     # (S, RD)
        sin_g = sin[pid[b]]
        cosT = cos_g.T[DIMS_PERM]               # (64, S)
        sinT = sin_g.T[DIMS_PERM]
        csq = np.ascontiguousarray(np.vstack([cosT, cosT])).astype(NPBF)
        ssq = np.ascontiguousarray(np.vstack([sinT * sign, sinT * sign])).astype(NPBF)

        hsT = hs[b].T                           # (H, S)
        hs_t = np.ascontiguousarray(
            hsT.reshape(KH, 128, S).transpose(1, 0, 2)).astype(NPBF)

        per_core.append({
            "hs_d": hs_t,
            "wa_d": wa_t_par[t],
            "wqb_d": wqb_t,
            "wk_d": wk_t,
            "wv_d": wv_t,
            "wo_d": wo_t,
            "csq_d": csq,
            "ssq_d": ssq,
            "mask_d": maskL.astype(NPBF),
            "id_d": ident.astype(NPBF),
            "ones_d": np.ones((128, 1), NPBF),
        })
    return per_core


def kernel(**inputs):
    if "nc" not in _NC_CACHE:
        _NC_CACHE["nc"] = _build_nc()
    nc = _NC_CACHE["nc"]
    in_maps = _host_prep(inputs)
    res = bass_utils.run_bass_kernel_spmd(nc, in_maps, core_ids=list(range(NCORES)))
    outs = []
    for b in range(B):
        acc = res.results[TP * b]["outT"].astype(np.float32)
        for t in range(1, TP):
            acc = acc + res.results[TP * b + t]["outT"]
        outs.append(acc.T)
    return np.stack(outs, axis=0)


# revision 48
# speedup vs baseline: 1.0574x; 1.0161x over previous
"""MLA (multi-headed latent attention) forward on 8 Trainium2 NeuronCores.

Sharding: data-parallel over batch (4) x tensor-parallel over heads (2):
core c handles batch c//2 with heads [16*(c%2), 16*(c%2)+16).
Each core computes a partial (H-dim) output contribution; host sums the
TP pair and stacks batches.

All matmuls run in bf16 (fp32 PSUM accumulation). Weights are host-
pre-tiled into the exact SBUF layouts so every DMA is contiguous.
LN means are folded into the fused projection as extra weight columns.

Schedule (the point of this structure is to keep the PE gap-free so the
HAM clock gate stays at 2.4 GHz):
- phase-1 m-tile order pe, qa x6, kv x4 so both TP AllGathers launch
  early and complete while the PE chews the kv tiles;
- mean subtraction happens on the raw bf16 projections as soon as the
  inputs exist; the qa rstd is folded into the qb-projection copies so
  the sqrt chain is never on the PE critical path;
- the causal mask is a third accumulating matmul on diagonal score
  blocks (exp of -30000 -> 0), removing the vector hop from the
  softmax chain: PE -> Act -> PE;
- the two heads of each group are interleaved block-by-block and
  sum/pv matmuls are software-pipelined one block behind the scores,
  so there is always independent PE work while Act computes exp.
Layout is feature-major ("T" = [feature, token]) everywhere except v
(token-major for the PV matmul).
"""

import numpy as np
import ml_dtypes
import concourse.bass as bass
import concourse.mybir as mybir
import concourse.tile as tile
from concourse import bacc
from concourse import bass_utils

F32 = mybir.dt.float32
BF16 = mybir.dt.bfloat16
AX = mybir.AxisListType
OP = mybir.AluOpType
AF = mybir.ActivationFunctionType
NPBF = ml_dtypes.bfloat16

B, S, H, NH = 4, 1024, 4096, 32
QL, KVL, RD, ND, VD = 1536, 512, 64, 128, 128
QHD = ND + RD  # 192
EPS = 1e-6
NCORES = 8
TP = 2                 # tensor-parallel ways (heads)
HPC = NH // TP         # 16 heads per core
G = 2                  # heads per group
NG = HPC // G          # 8 groups
TOKT = S // 128        # 8 token tiles
KH = H // 128          # 32 contraction tiles for H
NMT = 9                # phase-1 m-tiles/core: 1 pe+stats, 6 own qa, 2 own kv
OWN = 6                # qa m-tiles computed locally; partner half via AllGather
KVOWN = 2              # kv m-tiles computed locally; partner half via AllGather
CC_GROUPS = [[0, 1], [2, 3], [4, 5], [6, 7]]
SCALE = float(QHD) ** -0.5
MASKV = -30000.0

# rope feature permutation: pairs (d, d+32) land 16 lanes apart within a
# 32-partition quadrant so stream_shuffle can do rotate_half.
DIMS_PERM = np.array(
    list(range(0, 16)) + list(range(32, 48))
    + list(range(16, 32)) + list(range(48, 64)), dtype=np.int64)
SHUF_MASK = [(i + 16) % 32 for i in range(32)]

_NC_CACHE = {}


def _build_nc():
    nc = bacc.Bacc("TRN2", target_bir_lowering=False, debug=False)

    hs_d = nc.dram_tensor("hs_d", (128, KH, S), BF16, kind="ExternalInput").ap()
    wa_d = nc.dram_tensor("wa_d", (128, NMT, KH, 128), BF16, kind="ExternalInput").ap()
    wqb_d = nc.dram_tensor("wqb_d", (128, NG, 3, 12, 128), BF16, kind="ExternalInput").ap()
    wk_d = nc.dram_tensor("wk_d", (128, NG, 2, 4, 128), BF16, kind="ExternalInput").ap()
    wv_d = nc.dram_tensor("wv_d", (128, NG, 4, 256), BF16, kind="ExternalInput").ap()
    wo_d = nc.dram_tensor("wo_d", (128, KH, HPC, 128), BF16, kind="ExternalInput").ap()
    csq_d = nc.dram_tensor("csq_d", (128, S), BF16, kind="ExternalInput").ap()
    ssq_d = nc.dram_tensor("ssq_d", (128, S), BF16, kind="ExternalInput").ap()
    mask_d = nc.dram_tensor("mask_d", (128, 128), BF16, kind="ExternalInput").ap()
    id_d = nc.dram_tensor("id_d", (128, 128), BF16, kind="ExternalInput").ap()
    ones_d = nc.dram_tensor("ones_d", (128, 1), BF16, kind="ExternalInput").ap()
    outT = nc.dram_tensor("outT", (H, S), F32, kind="ExternalOutput").ap()
    stage_a = nc.dram_tensor("stage_a", (512, S), BF16, kind="Internal").ap()
    stage_b = nc.dram_tensor("stage_b", (258, S), BF16, kind="Internal").ap()
    stage_k = nc.dram_tensor("stage_k", (258, S), BF16, kind="Internal").ap()
    gath_a = nc.dram_tensor("gath_a", (2, 512, S), BF16, kind="Internal").ap()
    gath_b = nc.dram_tensor("gath_b", (2, 258, S), BF16, kind="Internal").ap()
    gath_k = nc.dram_tensor("gath_k", (2, 258, S), BF16, kind="Internal").ap()

    with tile.TileContext(nc) as tc:
        with tc.tile_pool(name="pers", bufs=1) as pers:
            # ---------------- persistent tiles ----------------
            q_anT = pers.tile([128, 12 * S], BF16)     # (q_a - m)^T  (1536, 1024)
            kv_cnT = pers.tile([128, 4 * S], BF16)     # LN(kv_c)^T (512, 1024)
            kpeT2 = pers.tile([128, S], BF16)          # roped k_pe^T, both halves
            csq_t = pers.tile([128, S], BF16)
            ssq_t = pers.tile([128, S], BF16)
            mask_t = pers.tile([128, 128], BF16)
            id_t = pers.tile([128, 128], BF16)
            ones_t = pers.tile([128, 1], BF16)
            # LN stat rows: all at partition 0 (engine base-partition rules
            # and the Q7 broadcast assume it); the prow tiles double as the
            # rstd-chain scratch once their add has consumed them.
            mrow_qa = pers.tile([1, S], F32, name="mrow_qa")
            mrow_kv = pers.tile([1, S], F32, name="mrow_kv")
            sqrow_qa = pers.tile([1, S], F32, name="sqrow_qa")
            sqrow_kv = pers.tile([1, S], F32, name="sqrow_kv")
            prow_qa = pers.tile([1, S], F32, name="prow_qa")
            prow_kv = pers.tile([1, S], F32, name="prow_kv")
            mb_qa = pers.tile([128, S], BF16, name="mb_qa")    # mean bcast
            mb_kv = pers.tile([128, S], BF16, name="mb_kv")
            rb_qa = pers.tile([128, S], F32, name="rb_qa")     # rstd bcast
            rb_kv = pers.tile([128, S], F32, name="rb_kv")

            # ones first (tiny, needed by the stats matmuls); mask/id via
            # the software-DGE pool queue (needed only by attention).
            nc.scalar.dma_start(out=ones_t[:, :], in_=ones_d)
            nc.gpsimd.dma_start(out=mask_t[:, :], in_=mask_d)
            nc.gpsimd.dma_start(out=id_t[:, :], in_=id_d)

            def rstd_chain(v1, sqrow, mrow, rb):
                # rows ([1,S]): var = E[x^2] - m^2; rstd = 1/sqrt(var+eps)
                nc.vector.tensor_tensor(out=v1[:, :], in0=mrow[:, :],
                                        in1=mrow[:, :], op=OP.mult)
                nc.vector.tensor_tensor(out=v1[:, :], in0=sqrow[:, :],
                                        in1=v1[:, :], op=OP.subtract)
                nc.vector.tensor_scalar_add(v1[:, :], v1[:, :], EPS)
                nc.scalar.sqrt(v1[:, :], v1[:, :])
                nc.vector.reciprocal_approx_fast(out=v1[:, :], in_=v1[:, :])
                nc.gpsimd.partition_broadcast(rb[:, :], v1[:, :])

            # ======== phase 1 (feature-major): X^T = Wa^T @ hs^T ====
            # m-tile order: the exchanged qa tiles first so both qa gathers
            # launch as early as possible (their latency hides under pe/kv
            # and the phase-2 qb matmuls, which need only the qa exchange);
            # pe next (LN means), own kv last (kv gather hides under the
            # qb work).
            M_TILES = ([("qa", i) for i in range(OWN)] + [("pe", 0)]
                       + [("kv", i) for i in range(KVOWN)])

            with tc.tile_pool(name="hsp", bufs=1) as hsp, \
                 tc.tile_pool(name="p1wa", bufs=3) as p1wa, \
                 tc.tile_pool(name="sqp", bufs=2) as sqp, \
                 tc.tile_pool(name="rowp", bufs=1) as rowp, \
                 tc.tile_pool(name="p1ps", bufs=3, space="PSUM") as p1ps, \
                 tc.tile_pool(name="stps", bufs=1, space="PSUM") as stps:
                hst = [hsp.tile([128, 4, S], BF16, name=f"hst_{i}")
                       for i in range(8)]
                # evens on the Act queue, odds interleaved between the early
                # weight tiles on the SP queue; cos/sin trail the Act stream.
                for i in (0, 2, 4, 6):
                    nc.scalar.dma_start(out=hst[i][:, :, :],
                                        in_=hs_d[:, 4 * i:4 * (i + 1), :])
                nc.scalar.dma_start(out=csq_t[:, :], in_=csq_d)
                nc.scalar.dma_start(out=ssq_t[:, :], in_=ssq_d)

                # sum-of-squares stats accumulators (fp32 PSUM)
                stat = {("qa", 0): stps.tile([1, 512], F32, name="st_qa0"),
                        ("qa", 1): stps.tile([1, 512], F32, name="st_qa1"),
                        ("kv", 0): stps.tile([1, 512], F32, name="st_kv0"),
                        ("kv", 1): stps.tile([1, 512], F32, name="st_kv1")}

                def mean_bcast(kind, mrow, mb):
                    # f32 mean row -> bf16 row -> [128,S] broadcast
                    r16 = rowp.tile([1, S], BF16, name=f"r16_{kind}")
                    nc.vector.tensor_scalar_mul(r16[:, :], mrow[:, :], 1.0)
                    nc.gpsimd.partition_broadcast(mb[:, :], r16[:, :])

                def tile_epilogue(kind, mi, dest):
                    # Square + stats matmuls + staging for one m-tile.
                    # Deferred one m-tile so the in-order PE queue never
                    # waits on the Act-queue Square before the next chains.
                    nmt = OWN if kind == "qa" else KVOWN
                    sqt = sqp.tile([128, S], BF16, tag="sq")
                    nc.scalar.activation(sqt[:, :], dest, AF.Square)
                    for qh in range(2):
                        sl = slice(qh * 512, qh * 512 + 512)
                        nc.tensor.matmul(
                            stat[(kind, qh)][:, :], ones_t[:, :], sqt[:, sl],
                            start=(mi == 0), stop=(mi == nmt - 1))
                    if kind == "qa":
                        # stage raw tiles for the TP-pair exchange
                        stg = stage_a if mi < 4 else stage_b
                        ro = (mi if mi < 4 else mi - 4) * 128
                        nc.sync.dma_start(out=stg[ro:ro + 128, :], in_=dest)
                        if mi == 3:
                            nc.gpsimd.collective_compute(
                                "AllGather", OP.bypass,
                                replica_groups=CC_GROUPS,
                                ins=[stage_a], outs=[gath_a])
                        if mi == OWN - 1:
                            # own E[x^2] rows, staged bitcast + second gather
                            for qh in range(2):
                                sl = slice(qh * 512, qh * 512 + 512)
                                nc.vector.tensor_scalar_mul(
                                    sqrow_qa[:, sl], stat[("qa", qh)][:, :],
                                    1.0 / QL)
                            sq16 = sqrow_qa[:, :].bitcast(BF16)  # [1, 2048]
                            nc.sync.dma_start(out=stage_b[256:257, :],
                                              in_=sq16[:, 0:S])
                            nc.sync.dma_start(out=stage_b[257:258, :],
                                              in_=sq16[:, S:2 * S])
                            nc.gpsimd.collective_compute(
                                "AllGather", OP.bypass,
                                replica_groups=CC_GROUPS,
                                ins=[stage_b], outs=[gath_b])
                    else:  # kv: stage for exchange, mean-subtract in place
                        nc.sync.dma_start(
                            out=stage_k[mi * 128:(mi + 1) * 128, :], in_=dest)
                        if mi == KVOWN - 1:
                            for qh in range(2):
                                sl = slice(qh * 512, qh * 512 + 512)
                                nc.vector.tensor_scalar_mul(
                                    sqrow_kv[:, sl], stat[("kv", qh)][:, :],
                                    1.0 / KVL)
                            sqk16 = sqrow_kv[:, :].bitcast(BF16)
                            nc.sync.dma_start(out=stage_k[256:257, :],
                                              in_=sqk16[:, 0:S])
                            nc.sync.dma_start(out=stage_k[257:258, :],
                                              in_=sqk16[:, S:2 * S])
                            nc.gpsimd.collective_compute(
                                "AllGather", OP.bypass,
                                replica_groups=CC_GROUPS,
                                ins=[stage_k], outs=[gath_k])
                        nc.vector.tensor_tensor(out=dest, in0=dest,
                                                in1=mb_kv[:, :], op=OP.subtract)

                first_sync_hs = [False]
                pending = None

                pid = nc.sync.partition_id()
                partner = 1 - (pid % 2)

                for (kind, mi) in M_TILES:
                    tix = {"pe": 0, "qa": 1, "kv": 7}[kind] + mi
                    wt = p1wa.tile([128, KH, 128], BF16, tag="wa",
                                   name=f"wa_{kind}_{mi}")
                    nc.sync.dma_start(out=wt[:, :, :], in_=wa_d[:, tix, :, :])
                    if not first_sync_hs[0]:
                        # odd hs tiles follow the first weight tile on sync;
                        # they must all be emitted before the first m-tile's
                        # matmuls (deps follow emission order)
                        first_sync_hs[0] = True
                        for i in (1, 3, 5, 7):
                            nc.sync.dma_start(
                                out=hst[i][:, :, :],
                                in_=hs_d[:, 4 * i:4 * (i + 1), :])
                        # force the partition-id register load on the SP
                        # engine now (a dynamic-offset 4-byte read) so the
                        # partner fetches later don't pay the DRAM load
                        nc.sync.dma_start(
                            out=prow_qa[0:1, 0:1],
                            in_=gath_b[partner, 256:257, 0:2].bitcast(F32))
                    if kind == "qa":
                        dest = q_anT[:, mi * S:(mi + 1) * S]
                    elif kind == "kv":
                        dest = kv_cnT[:, mi * S:(mi + 1) * S]
                    else:
                        dest = kpeT2[0:64, :]
                    for qh in range(2):
                        sl = slice(qh * 512, qh * 512 + 512)
                        ps = p1ps.tile([128, 512], F32, tag="p1")
                        for k in range(KH):
                            nc.tensor.matmul(
                                ps[:, :], wt[:, k, :], hst[k // 4][:, k % 4, sl],
                                start=(k == 0), stop=(k == KH - 1))
                        if kind == "pe":
                            nc.scalar.copy(dest[:, sl], ps[0:64, :])
                            # rows 64/96 carry sum(qa), sum(kv) over features
                            nc.vector.tensor_scalar_mul(
                                mrow_qa[:, sl], ps[64:65, :], 1.0 / QL)
                            nc.vector.tensor_scalar_mul(
                                mrow_kv[:, sl], ps[96:97, :], 1.0 / KVL)
                        else:
                            nc.scalar.copy(dest[:, sl], ps[:, :])
                    if kind == "pe":
                        # means -> bf16 broadcasts (early: deps ready now)
                        mean_bcast("qa", mrow_qa, mb_qa)
                        mean_bcast("kv", mrow_kv, mb_kv)
                        # mean-subtract the own qa slices (rstd is folded
                        # into the qb copies later); must be emitted after
                        # mb_qa's producer — deps follow emission order
                        for mj in range(OWN):
                            dsl = q_anT[:, mj * S:(mj + 1) * S]
                            nc.vector.tensor_tensor(
                                out=dsl, in0=dsl, in1=mb_qa[:, :],
                                op=OP.subtract)
                        # rope k_pe rows 0:64, duplicate into 64:128 (the
                        # duplicate is a software-DGE copy; slack is huge)
                        kp_sh = sqp.tile([64, S], BF16, tag="kpsh", name="kpsh")
                        nc.vector.stream_shuffle(
                            kp_sh[:, :].bitcast(F32), kpeT2[0:64, :].bitcast(F32),
                            SHUF_MASK)
                        nc.vector.tensor_tensor(out=kp_sh[:, :], in0=kp_sh[:, :],
                                                in1=ssq_t[:64, :], op=OP.mult)
                        nc.vector.tensor_tensor(out=kpeT2[0:64, :],
                                                in0=kpeT2[0:64, :],
                                                in1=csq_t[:64, :], op=OP.mult)
                        nc.vector.tensor_tensor(out=kpeT2[0:64, :],
                                                in0=kpeT2[0:64, :],
                                                in1=kp_sh[:, :], op=OP.add)
                        nc.gpsimd.dma_start(out=kpeT2[64:128, :],
                                            in_=kpeT2[0:64, :])
                        continue
                    if pending is not None:
                        tile_epilogue(*pending)
                        pending = None
                    if (kind == "kv" and mi == KVOWN - 1) or \
                            (kind == "qa" and mi == OWN - 1):
                        # flush inline: the gather for this kind must launch
                        # now, not one tile later
                        tile_epilogue(kind, mi, dest)
                    else:
                        pending = (kind, mi, dest)
                if pending is not None:
                    tile_epilogue(*pending)

            # ======== phase 2 + 3 ========
            with tc.tile_pool(name="otp", bufs=1) as otp:
                oT = otp.tile([128, HPC * S], BF16)    # normalized o^T
                with tc.tile_pool(name="gq2", bufs=3) as gqp, \
                     tc.tile_pool(name="wqp", bufs=2) as wqp, \
                     tc.tile_pool(name="wop", bufs=2) as wop, \
                     tc.tile_pool(name="op", bufs=2) as outp, \
                     tc.tile_pool(name="gkv", bufs=3) as gkvp, \
                     tc.tile_pool(name="wk", bufs=3) as wkp, \
                     tc.tile_pool(name="wv", bufs=2) as wvp, \
                     tc.tile_pool(name="rshp", bufs=1) as rshp, \
                     tc.tile_pool(name="pp", bufs=2) as ppool, \
                     tc.tile_pool(name="denp", bufs=1) as denp, \
                     tc.tile_pool(name="pjps", bufs=2, space="PSUM") as pjps, \
                     tc.tile_pool(name="sps", bufs=1, space="PSUM") as sps, \
                     tc.tile_pool(name="ops", bufs=1, space="PSUM") as ops, \
                     tc.tile_pool(name="smps", bufs=1, space="PSUM") as smps:

                    def qT_dma(g):
                        wt = wqp.tile([128, 3, 12, 128], BF16, tag="wqb",
                                      name=f"wqb_{g}")
                        nc.sync.dma_start(out=wt[:, :, :, :],
                                          in_=wqb_d[:, g, :, :, :])
                        return wt

                    def qT_mm(g, wt):
                        # q^T for this group: 2 nope m-tiles + 1 pe pair;
                        # the qa rstd is folded into the PSUM->SBUF copies.
                        qT = gqp.tile([128, 3 * S], BF16, tag="qT",
                                      name=f"qT_{g}")
                        for m in range(3):
                            for qh in range(2):
                                sl = slice(qh * 512, qh * 512 + 512)
                                ps = pjps.tile([128, 512], F32, tag="pj")
                                for k in range(12):
                                    nc.tensor.matmul(
                                        ps[:, :], wt[:, m, k, :],
                                        q_anT[:, k * S + qh * 512:
                                              k * S + qh * 512 + 512],
                                        start=(k == 0), stop=(k == 11))
                                nc.vector.tensor_tensor(
                                    out=qT[:, m * S + qh * 512:
                                           m * S + qh * 512 + 512],
                                    in0=ps[:, :], in1=rb_qa[:, sl], op=OP.mult)
                        # rope the pe tile (m=2): rows 0:64 head0, 64:128 head1
                        pe = qT[:, 2 * S:3 * S]
                        rsh = rshp.tile([128, S], BF16, tag="rsh")
                        nc.vector.stream_shuffle(
                            rsh[:, :].bitcast(F32), pe.bitcast(F32), SHUF_MASK)
                        nc.vector.tensor_tensor(out=rsh[:, :], in0=rsh[:, :],
                                                in1=ssq_t[:, :], op=OP.mult)
                        nc.vector.tensor_tensor(out=pe, in0=pe,
                                                in1=csq_t[:, :], op=OP.mult)
                        nc.vector.tensor_tensor(out=pe, in0=pe, in1=rsh[:, :],
                                                op=OP.add)
                        return qT

                    def emit_knv(g):
                        # k_nope^T (2 m-tiles) and v (token-major)
                        knT = gkvp.tile([128, 2 * S], BF16, tag="knT",
                                        name=f"knT_{g}")
                        for m in range(2):
                            wt = wkp.tile([128, 4, 128], BF16, tag="wk",
                                          name=f"wk_{g}_{m}")
                            nc.sync.dma_start(out=wt[:, :, :],
                                              in_=wk_d[:, g, m, :, :])
                            for qh in range(2):
                                ps = pjps.tile([128, 512], F32, tag="pj")
                                for k in range(4):
                                    nc.tensor.matmul(
                                        ps[:, :], wt[:, k, :],
                                        kv_cnT[:, k * S + qh * 512:
                                               k * S + qh * 512 + 512],
                                        start=(k == 0), stop=(k == 3))
                                nc.scalar.copy(knT[:, m * S + qh * 512:
                                                   m * S + qh * 512 + 512],
                                               ps[:, :])

                        v_sb = gkvp.tile([128, TOKT * G * VD], BF16, tag="v",
                                         name=f"v_{g}")
                        wv_t = wvp.tile([128, 4, 256], BF16, tag="wv",
                                        name=f"wv_{g}")
                        nc.sync.dma_start(out=wv_t[:, :, :], in_=wv_d[:, g, :, :])
                        for t in range(TOKT):
                            ps = pjps.tile([128, 512], F32, tag="pj")
                            for k in range(4):
                                nc.tensor.matmul(
                                    ps[:, :256],
                                    kv_cnT[:, k * S + t * 128:
                                           k * S + (t + 1) * 128],
                                    wv_t[:, k, :], start=(k == 0), stop=(k == 3))
                            nc.vector.tensor_scalar_mul(
                                v_sb[:, t * 256:(t + 1) * 256], ps[:, :256], 1.0)
                        return knT, v_sb

                    def wo_dma(hr):
                        wt = wop.tile([128, HPC, 128], BF16, tag="wo",
                                      name=f"wo_{hr}")
                        nc.sync.dma_start(out=wt[:, :, :], in_=wo_d[:, hr, :, :])
                        return wt

                    def qa_finalize():
                        # partner qa tiles + stats merge (waits on the
                        # second gather; hidden under the knv matmuls)
                        nc.sync.dma_start(
                            out=q_anT[:, OWN * S:10 * S]
                                .rearrange("p (k t) -> p k t", k=4),
                            in_=gath_a[partner, 0:512, :]
                                .rearrange("(k p) t -> p k t", p=128))
                        nc.sync.dma_start(
                            out=q_anT[:, 10 * S:12 * S]
                                .rearrange("p (k t) -> p k t", k=2),
                            in_=gath_b[partner, 0:256, :]
                                .rearrange("(k p) t -> p k t", p=128))
                        nc.sync.dma_start(
                            out=prow_qa[:, :],
                            in_=gath_b[partner, 256:258, :].bitcast(F32))
                        for mj in range(OWN, 12):
                            dsl = q_anT[:, mj * S:(mj + 1) * S]
                            nc.vector.tensor_tensor(
                                out=dsl, in0=dsl, in1=mb_qa[:, :],
                                op=OP.subtract)
                        nc.vector.tensor_tensor(
                            out=sqrow_qa[:, :], in0=sqrow_qa[:, :],
                            in1=prow_qa[:, :], op=OP.add)
                        rstd_chain(prow_qa, sqrow_qa, mrow_qa, rb_qa)

                    def kv_finalize():
                        # partner kv tiles + stats merge; rstd applied to
                        # all four slots (deadline: the knv matmuls)
                        nc.sync.dma_start(
                            out=kv_cnT[:, KVOWN * S:4 * S]
                                .rearrange("p (k t) -> p k t", k=2),
                            in_=gath_k[partner, 0:256, :]
                                .rearrange("(k p) t -> p k t", p=128))
                        nc.sync.dma_start(
                            out=prow_kv[:, :],
                            in_=gath_k[partner, 256:258, :].bitcast(F32))
                        for mj in range(KVOWN, 4):
                            dsl = kv_cnT[:, mj * S:(mj + 1) * S]
                            nc.vector.tensor_tensor(
                                out=dsl, in0=dsl, in1=mb_kv[:, :],
                                op=OP.subtract)
                        nc.vector.tensor_tensor(
                            out=sqrow_kv[:, :], in0=sqrow_kv[:, :],
                            in1=prow_kv[:, :], op=OP.add)
                        rstd_chain(prow_kv, sqrow_kv, mrow_kv, rb_kv)
                        for mj in range(4):
                            dsl = kv_cnT[:, mj * S:(mj + 1) * S]
                            nc.vector.tensor_tensor(
                                out=dsl, in0=dsl, in1=rb_kv[:, :], op=OP.mult)

                    # order: the qb projections first (they need only the qa
                    # exchange, which launched early); the kv exchange and
                    # its LN finalize hide under them, then knv.
                    qa_finalize()
                    wq_t = {0: qT_dma(0), 1: qT_dma(1)}
                    qts = {0: qT_mm(0, wq_t.pop(0))}
                    wq_t[2] = qT_dma(2)
                    qts[1] = qT_mm(1, wq_t.pop(1))
                    qts[2] = qT_mm(2, wq_t.pop(2))
                    wq_t[3] = qT_dma(3)
                    kv_finalize()
                    knv = {}
                    for gg in range(3):
                        knv[gg] = emit_knv(gg)
                    wo_pre = []

                    for g in range(NG):
                        knT, v_sb = knv.pop(g)
                        qT = qts.pop(g)
                        if g == NG - 1:
                            wo_pre = [wo_dma(0), wo_dma(1)]

                        # ---- attention: heads interleaved per block,
                        # sum/pv pipelined one block behind the scores ----
                        for qh in range(2):
                            nik = 4 * (qh + 1)
                            po = [ops.tile([128, 512], F32, tag=f"po{hh}",
                                           name=f"po_{g}_{qh}_{hh}")
                                  for hh in range(G)]
                            psm = [smps.tile([1, 512], F32, tag=f"pm{hh}",
                                             name=f"pm_{g}_{qh}_{hh}")
                                   for hh in range(G)]

                            def scores(ik):
                                lo = max(128 * ik, 512 * qh)
                                hi = 512 * (qh + 1)
                                w = hi - lo
                                diag = (lo == 128 * ik)
                                p_l = []
                                for hh in range(G):
                                    ps_s = sps.tile([128, 512], F32,
                                                    tag=f"ps{hh}")
                                    nc.tensor.matmul(
                                        ps_s[:, :w],
                                        knT[:, hh * S + ik * 128:
                                            hh * S + (ik + 1) * 128],
                                        qT[:, hh * S + lo: hh * S + hi],
                                        start=True, stop=False)
                                    if diag:
                                        nc.tensor.matmul(
                                            ps_s[:, 0:128], id_t[:, :],
                                            mask_t[:, :],
                                            start=False, stop=False)
                                    nc.tensor.matmul(
                                        ps_s[:, :w],
                                        kpeT2[hh * 64:(hh + 1) * 64,
                                              ik * 128:(ik + 1) * 128],
                                        qT[hh * 64:(hh + 1) * 64,
                                           2 * S + lo: 2 * S + hi],
                                        start=False, stop=True)
                                    p = ppool.tile([128, 512], BF16,
                                                   tag=f"p{hh}")
                                    nc.scalar.activation(p[:, :w], ps_s[:, :w],
                                                         AF.Exp, scale=SCALE)
                                    p_l.append((p, w, lo))
                                return p_l

                            def sum_pv(ik, p_l):
                                for hh in range(G):
                                    p, w, lo = p_l[hh]
                                    osl = slice(lo - 512 * qh, hi_q - 512 * qh)
                                    nc.tensor.matmul(
                                        psm[hh][:, osl],
                                        ones_t[:, :], p[:, :w],
                                        start=(ik == 0), stop=(ik == nik - 1))
                                    nc.tensor.matmul(
                                        po[hh][:, osl],
                                        v_sb[:, ik * 256 + hh * 128:
                                             ik * 256 + (hh + 1) * 128],
                                        p[:, :w],
                                        start=(ik == 0), stop=(ik == nik - 1))

                            hi_q = 512 * (qh + 1)
                            prev = None
                            for ik in range(nik):
                                p_l = scores(ik)
                                if prev is not None:
                                    sum_pv(prev[0], prev[1])
                                prev = (ik, p_l)
                            sum_pv(prev[0], prev[1])

                            for hh in range(G):
                                hg = g * G + hh
                                srow = denp.tile([1, 512], F32, tag=f"dr{hh}")
                                nc.scalar.copy(srow[:, :], psm[hh][:, :])
                                rbc = denp.tile([128, 512], F32, tag=f"db{hh}")
                                nc.gpsimd.partition_broadcast(rbc[:, :],
                                                              srow[:, :])
                                nc.vector.reciprocal_approx_fast(
                                    out=rbc[:, :], in_=rbc[:, :])
                                nc.vector.tensor_tensor(
                                    out=oT[:, hg * S + qh * 512:
                                           hg * S + qh * 512 + 512],
                                    in0=po[hh][:, :], in1=rbc[:, :],
                                    op=OP.mult)

                        if g + 3 < NG:
                            knv[g + 3] = emit_knv(g + 3)
                        if g + 3 < NG:
                            qts[g + 3] = qT_mm(g + 3, wq_t.pop(g + 3))
                        if g + 4 < NG:
                            wq_t[g + 4] = qT_dma(g + 4)

                    # ======== phase 3: out^T = Wo^T @ o ========
                    for hr in range(KH):
                        wt = wo_pre[hr] if hr < len(wo_pre) else wo_dma(hr)
                        ot = outp.tile([128, S], F32, tag="out")
                        for qh in range(2):
                            sl = slice(qh * 512, qh * 512 + 512)
                            ps = sps.tile([128, 512], F32, tag=f"ps{qh}")
                            for m in range(HPC):
                                nc.tensor.matmul(
                                    ps[:, :], wt[:, m, :],
                                    oT[:, m * S + qh * 512:
                                       m * S + qh * 512 + 512],
                                    start=(m == 0), stop=(m == HPC - 1))
                            if qh == 0:
                                nc.scalar.copy(ot[:, sl], ps[:, :])
                            else:
                                nc.vector.tensor_scalar_mul(ot[:, sl], ps[:, :], 1.0)
                        nc.sync.dma_start(
                            out=outT[hr * 128:(hr + 1) * 128, :], in_=ot[:, :])
    nc.compile()
    return nc


def _host_prep(inputs):
    hs = np.asarray(inputs["hidden_states"], np.float32)
    cos = np.asarray(inputs["cos"], np.float32)
    sin = np.asarray(inputs["sin"], np.float32)
    pid = np.asarray(inputs["position_ids"]).astype(np.int64)
    Wqa = np.asarray(inputs["Wqa"], np.float32)
    gqa = np.asarray(inputs["gqa"], np.float32)
    Wqb = np.asarray(inputs["Wqb"], np.float32)
    Wkva = np.asarray(inputs["Wkva"], np.float32)
    gkva = np.asarray(inputs["gkva"], np.float32)
    Wkvb = np.asarray(inputs["Wkvb"], np.float32)
    Wo = np.asarray(inputs["Wo"], np.float32)

    # phase-1 fused projection: [pe'+sums | qa x6 | kv x4] m-tiles.
    # pe tile cols: 0:64 rope-permuted Wkva-pe, 64 sum(Wqa cols),
    # 96 sum(Wkva kv cols) — yields feature-sum rows for the LN means.
    wsum_qa = Wqa.sum(axis=1, keepdims=True)
    wsum_kv = Wkva[:, :KVL].sum(axis=1, keepdims=True)
    pe_cols = np.concatenate(
        [Wkva[:, KVL:][:, DIMS_PERM], wsum_qa, np.zeros((H, 31), np.float32),
         wsum_kv, np.zeros((H, 31), np.float32)], axis=1)
    # per-parity fused projection: shared [pe] + own qa half (6 tiles)
    # + own kv half (2 tiles)
    wa_t_par = []
    for t in range(TP):
        wa = np.concatenate(
            [pe_cols, Wqa[:, t * OWN * 128:(t + 1) * OWN * 128],
             Wkva[:, t * KVOWN * 128:KVL][:, :KVOWN * 128]], axis=1)
        wa_t_par.append(np.ascontiguousarray(
            wa.reshape(KH, 128, NMT, 128).transpose(1, 2, 0, 3)).astype(NPBF))

    # fold LN gains into the B-projections (bias terms are zero per spec)
    Wqb = Wqb * gqa[:, None]
    Wkvb = Wkvb * gkva[:, None]

    # sign pattern for the shuffle-based rotate_half
    sign = np.where(DIMS_PERM < RD // 2, -1.0, 1.0).astype(np.float32)[:, None]

    kp, q = np.mgrid[0:128, 0:128]
    maskL = np.where(q < kp, MASKV, 0.0).astype(np.float32)
    ident = np.eye(128, dtype=np.float32)

    per_core = []
    w4 = Wqb.reshape(QL, NH, QHD)
    wk4 = Wkvb.reshape(KVL, NH, ND + VD)
    for c in range(NCORES):
        b, t = divmod(c, TP)
        heads = slice(t * HPC, (t + 1) * HPC)
        # Wqb: group-blocked [h0 nope | h1 nope | h0 pe' | h1 pe'] per group
        wq = w4[:, heads]                       # (QL, 16, 192)
        nope = wq[:, :, :ND]                    # (QL, 16, 128)
        pe = wq[:, :, ND:][:, :, DIMS_PERM]     # (QL, 16, 64) permuted
        blocks = []
        for g in range(NG):
            blocks.extend([nope[:, 2 * g], nope[:, 2 * g + 1],
                           pe[:, 2 * g], pe[:, 2 * g + 1]])
        wqb_c = np.concatenate(blocks, axis=1)  # (QL, NG*384)
        # k-subtile order must match this core's q_anT slots: own half first
        kperm = list(range(t * OWN, t * OWN + OWN)) \
            + list(range((1 - t) * OWN, (1 - t) * OWN + OWN))
        # -> (128, NG, 3, 12, 128)
        wqb_t = np.ascontiguousarray(
            wqb_c.reshape(12, 128, NG, 3, 128)[kperm].transpose(1, 2, 3, 0, 4)
        ).astype(NPBF)

        # kv_cnT is own-tiles-first on each core; permute the contraction
        # axis of the decompression weights to match
        kperm_kv = [KVOWN * t, KVOWN * t + 1,
                    KVOWN * (1 - t), KVOWN * (1 - t) + 1]
        wkc = wk4[:, heads]
        wkvbk_c = wkc[:, :, :ND].reshape(KVL, HPC * ND)
        # -> (128, NG, 2, 4, 128)
        wk_t = np.ascontiguousarray(
            wkvbk_c.reshape(4, 128, NG, 2, 128)[kperm_kv]
            .transpose(1, 2, 3, 0, 4)
        ).astype(NPBF)
        wkvbv_c = wkc[:, :, ND:].reshape(KVL, HPC * VD)
        # -> (128, NG, 4, 256)
        wv_t = np.ascontiguousarray(
            wkvbv_c.reshape(4, 128, NG, 256)[kperm_kv].transpose(1, 2, 0, 3)
        ).astype(NPBF)

        wo_c = Wo[t * HPC * VD:(t + 1) * HPC * VD]   # (2048, 4096)
        # -> (128, 32, 16, 128)
        wo_t = np.ascontiguousarray(
            wo_c.reshape(HPC, 128, KH, 128).transpose(1, 2, 0, 3)).astype(NPBF)

        cos_g = cos[pid[b]]                     # (S, RD)
        sin_g = sin[pid[b]]
        cosT = cos_g.T[DIMS_PERM]               # (64, S)
        sinT = sin_g.T[DIMS_PERM]
        csq = np.ascontiguousarray(np.vstack([cosT, cosT])).astype(NPBF)
        ssq = np.ascontiguousarray(np.vstack([sinT * sign, sinT * sign])).astype(NPBF)

        hsT = hs[b].T                           # (H, S)
        hs_t = np.ascontiguousarray(
            hsT.reshape(KH, 128, S).transpose(1, 0, 2)).astype(NPBF)

        per_core.append({
            "hs_d": hs_t,
            "wa_d": wa_t_par[t],
            "wqb_d": wqb_t,
            "wk_d": wk_t,
            "wv_d": wv_t,
            "wo_d": wo_t,
            "csq_d": csq,
            "ssq_d": ssq,
            "mask_d": maskL.astype(NPBF),
            "id_d": ident.astype(NPBF),
            "ones_d": np.ones((128, 1), NPBF),
        })
    return per_core


def kernel(**inputs):
    if "nc" not in _NC_CACHE:
        _NC_CACHE["nc"] = _build_nc()
    nc = _NC_CACHE["nc"]
    in_maps = _host_prep(inputs)
    res = bass_utils.run_bass_kernel_spmd(nc, in_maps, core_ids=list(range(NCORES)))
    outs = []
    for b in range(B):
        acc = res.results[TP * b]["outT"].astype(np.float32)
        for t in range(1, TP):
            acc = acc + res.results[TP * b + t]["outT"]
        outs.append(acc.T)
    return np.stack(outs, axis=0)
